# revision 24
# baseline (speedup 1.0000x reference)
"""GCN encoder (4x GCNConv+ReLU+BatchNorm) as a Trainium2 Bass kernel on 8 cores.

Sharding: destination nodes are partitioned into 8*T degree-balanced buckets of
128 rows (one bucket = one 128-row "tile" on one core).  Per layer, each core:
  1. PE-transposes each z tile (fused with the BatchNorm apply of the previous
     layer), computes h = z @ W on PE, scales rows by dis = 1/sqrt(deg) -> g.
  2. AllGathers g into a full [NPAD, 128] DRAM table.
  3. For each dst tile, gathers the source rows of its (host-bucketed, padded)
     edges with SWDGE dma_gather, builds one-hot selector matrices on DVE
     (is_equal against an iota row) and segment-sums messages into PSUM with
     PE matmuls (out = S.T @ msgs).
  4. z' = relu(dis * (agg + g) + b); BN statistics via matmuls with a validity
     mask column; stats AllReduce'd across cores ([128, 2]).
The final BN apply happens in the transposed orientation, so the kernel output
is per-tile transposed; the host undoes the permutation and transposition.
"""

import math
import os
from contextlib import ExitStack

import numpy as np

import concourse.bass as bass
import concourse.bacc as bacc
import concourse.mybir as mybir
import concourse.tile as tile
from concourse import bass_utils

P = 128          # partitions / tile rows
D = 128          # feature dim
L = 4            # layers
C = 8            # cores
HALF = 32768     # int16 gather-index limit
EPS = 1e-5
F32 = mybir.dt.float32
BF16 = mybir.dt.bfloat16
I16 = mybir.dt.int16
ALU = mybir.AluOpType
GROUP_TILES = 8  # dst tiles per gather batch
ABLATE = set()   # debug: subset of {"agg", "gather", "stats", "allgather"}
SMOD = 0         # every SMOD-th selector build goes to ACT (0 = never; ACT latency makes this a loss)
NSWQ = 2         # SWDGE queues; gathers alternate queues to overlap desc-gen/transfer


# --------------------------------------------------------------------------
# Host-side sharding
# --------------------------------------------------------------------------

def _balance_buckets(deg, n_buckets):
    """Snake round-robin over degree-sorted nodes -> (bucket, slot) per node.
    Each bucket gets <= ceil(N / n_buckets) nodes with near-equal degree sums."""
    n = deg.shape[0]
    order = np.argsort(-deg, kind="stable")
    idx = np.arange(n)
    rounds = idx // n_buckets
    pos = idx % n_buckets
    b_of = np.where(rounds % 2 == 0, pos, n_buckets - 1 - pos)
    bucket = np.empty(n, np.int64)
    slot = np.empty(n, np.int64)
    bucket[order] = b_of
    slot[order] = rounds
    return bucket, slot


def _wrap_idx(v):
    """Wrap a flat int array into the SWDGE index image rows: img[p, s] =
    v[s*16 + p % 16], replicated across the 8 groups of 16 partitions."""
    n = v.shape[0]
    assert n % 16 == 0
    blk = v.reshape(n // 16, 16).T.astype(np.int16)   # [16, n/16]
    return np.tile(blk, (8, 1))                       # [128, n/16]


def prep(x, edge_index):
    """Shard the graph. Returns (meta, per_core_inputs: list of dicts)."""
    n_nodes, d = x.shape
    assert d == D
    src = np.asarray(edge_index[0], dtype=np.int64)
    dst = np.asarray(edge_index[1], dtype=np.int64)

    T = -(-n_nodes // (C * P))          # tiles per core
    R = T * P                           # padded rows per core
    NPAD = C * R
    assert NPAD - HALF < HALF, "second half-table must also fit int16 indices"

    deg = np.bincount(dst, minlength=n_nodes)
    bucket, slot = _balance_buckets(deg, C * T)
    assert slot.max() < P
    core_of = bucket // T
    tile_of = bucket % T
    pad_id = core_of * R + tile_of * P + slot        # padded global row id

    cnt = np.bincount(bucket, minlength=C * T).reshape(C, T)

    TS = -(-T // 2)                     # tiles per core in table X
    TSY = T - TS                        # tiles per core in table Y
    assert C * TS * P <= 32768 and C * TSY * P <= 32768

    e_core = core_of[dst]
    e_tile = tile_of[dst]
    e_off = slot[dst]
    src_core = core_of[src]
    src_tile = tile_of[src]
    src_slot = slot[src]
    e_half = (src_tile >= TS).astype(np.int64)    # 0 -> table X, 1 -> table Y
    e_srcp = np.where(e_half == 0,
                      src_core * (TS * P) + src_tile * P + src_slot,
                      src_core * (TSY * P) + (src_tile - TS) * P + src_slot)

    # chunk budgets per (tile, half): max over cores
    key = ((e_core * T) + e_tile) * 2 + e_half
    ecnt = np.bincount(key, minlength=C * T * 2).reshape(C, T, 2)
    CA = -(-ecnt[:, :, 0].max(axis=0) // P)          # [T]
    CB = -(-ecnt[:, :, 1].max(axis=0) // P)

    groups = [list(range(s, min(s + GROUP_TILES, T)))
              for s in range(0, T, GROUP_TILES)]

    # chunk column bases (t-major, A chunks then B chunks per tile)
    colA = np.zeros(T, np.int64)
    colB = np.zeros(T, np.int64)
    c = 0
    for t in range(T):
        colA[t] = c
        c += CA[t]
        colB[t] = c
        c += CB[t]
    NCH = int(c)

    # idx image column layout per group: A segment then B segment
    idx_base = []
    ic = 0
    for grp in groups:
        nA = int(sum(CA[t] for t in grp))
        nB = int(sum(CB[t] for t in grp))
        idx_base.append((ic, nA * 8, ic + nA * 8, nB * 8, nA, nB))
        ic += (nA + nB) * 8
    IDXC = ic
    MAXSLOTS = max(a[4] + a[5] for a in idx_base)

    meta = dict(N=n_nodes, T=T, R=R, NPAD=NPAD, CA=CA, CB=CB,
                groups=groups, colA=colA, colB=colB, NCH=NCH,
                idx_base=idx_base, IDXC=IDXC, MAXSLOTS=MAXSLOTS,
                pad_id=pad_id, TS=TS, TSY=TSY)

    # ---------------- per-core images ----------------
    per_core = []
    eorder = np.lexsort((e_half, e_tile, e_core))    # sort edges
    s_core = e_core[eorder]
    s_tile = e_tile[eorder]
    s_half = e_half[eorder]
    s_srcp = e_srcp[eorder]
    s_off = e_off[eorder]

    # start offset of each (core, tile, half) run in the sorted arrays
    runkey = ((s_core * T) + s_tile) * 2 + s_half
    starts = np.searchsorted(runkey, np.arange(C * T * 2))

    for ci in range(C):
        z0 = np.zeros((R, D), np.float32)
        nodes_c = np.where(core_of == ci)[0]
        z0[tile_of[nodes_c] * P + slot[nodes_c]] = x[nodes_c]

        degT = np.ones((P, T), np.float32)
        mskT = np.zeros((P, T), np.float32)
        tt = tile_of[nodes_c]
        ss = slot[nodes_c]
        degT[ss, tt] = deg[nodes_c] + 1.0
        mskT[ss, tt] = 1.0

        offimg = np.full((P, NCH), -1.0, np.float32)
        idx_img = np.zeros((P, IDXC), np.int16)

        for gi, grp in enumerate(groups):
            for hi in (0, 1):
                vs = []
                for t in grp:
                    budget = (CA[t] if hi == 0 else CB[t]) * P
                    if budget == 0:
                        continue
                    k0 = ((ci * T) + t) * 2 + hi
                    a = starts[k0]
                    b = starts[k0 + 1] if k0 + 1 < C * T * 2 else len(runkey)
                    srcs = s_srcp[a:b]
                    offs = s_off[a:b]
                    assert len(srcs) <= budget
                    v = np.zeros(budget, np.int64)
                    v[:len(srcs)] = srcs
                    o = np.full(budget, -1.0, np.float32)
                    o[:len(srcs)] = offs
                    vs.append(v)
                    cb = (colA[t] if hi == 0 else colB[t])
                    nch_t = budget // P
                    offimg[:, cb:cb + nch_t] = o.reshape(nch_t, P).T
                if vs:
                    vflat = np.concatenate(vs)
                    cs = idx_base[gi][0] if hi == 0 else idx_base[gi][2]
                    cw = idx_base[gi][1] if hi == 0 else idx_base[gi][3]
                    assert vflat.shape[0] // 16 == cw  # cols match
                    idx_img[:, cs:cs + cw] = _wrap_idx(vflat)

        per_core.append(dict(z0=z0, degT=degT, mskT=mskT,
                             idximg=idx_img, offimg=offimg))
    return meta, per_core


# --------------------------------------------------------------------------
# Device program
# --------------------------------------------------------------------------

def build(nc, meta):
    T = meta["T"]
    NPAD = meta["NPAD"]
    NN = meta["N"]
    CA, CB = meta["CA"], meta["CB"]
    colA, colB = meta["colA"], meta["colB"]
    groups = meta["groups"]
    idx_base = meta["idx_base"]
    IDXC, NCH, MAXSLOTS = meta["IDXC"], meta["NCH"], meta["MAXSLOTS"]

    z0_d = nc.dram_tensor("z0", [T * P, D], F32, kind="ExternalInput")
    w_d = nc.dram_tensor("wimg", [P, L * D], F32, kind="ExternalInput")
    brow_d = nc.dram_tensor("brow", [1, L * D], F32, kind="ExternalInput")
    gbt_d = nc.dram_tensor("gbt", [P, 2 * L], F32, kind="ExternalInput")
    deg_d = nc.dram_tensor("degT", [P, T], F32, kind="ExternalInput")
    msk_d = nc.dram_tensor("mskT", [P, T], F32, kind="ExternalInput")
    idx_d = nc.dram_tensor("idximg", [P, IDXC], I16, kind="ExternalInput")
    off_d = nc.dram_tensor("offimg", [P, NCH], F32, kind="ExternalInput")
    cst_d = nc.dram_tensor("consts", [P, 2 * P + 1], F32, kind="ExternalInput")
    zo_d = nc.dram_tensor("zout", [T * P, P], F32, kind="ExternalOutput")

    TS, TSY = meta["TS"], meta["TSY"]

    with tile.TileContext(nc) as tc, ExitStack() as ctx:
        dram = ctx.enter_context(tc.tile_pool(name="dram", bufs=1, space="DRAM"))
        g_ownX = dram.tile([TS * P, D], BF16)
        g_ownY = dram.tile([TSY * P, D], BF16)
        st_in = dram.tile([P, 2], F32)

        cpool = ctx.enter_context(tc.tile_pool(name="const", bufs=1))
        z_sb = cpool.tile([P, T * D], F32)
        g_sb = cpool.tile([P, T * D], F32)
        w_sb = cpool.tile([P, L * D], F32)
        brow_sb = cpool.tile([1, L * D], F32)
        brep_sb = cpool.tile([P, L * D], F32)
        gbt_sb = cpool.tile([P, 2 * L], F32)
        deg_sb = cpool.tile([P, T], F32)
        msk_sb = cpool.tile([P, T], F32)
        dis_sb = cpool.tile([P, T], F32)
        idx_sb = cpool.tile([P, IDXC], I16)
        off_sb = cpool.tile([P, NCH], F32)
        cst_sb = cpool.tile([P, 2 * P + 1], F32)
        iota_sb = cst_sb[:, 0:P]
        ident_sb = cst_sb[:, P:2 * P]
        eps_sb = cst_sb[:, 2 * P:2 * P + 1]
        ones_sb = cpool.tile([1, P], F32)
        negoff_sb = cpool.tile([P, NCH], F32)
        onecol_sb = cpool.tile([P, 1], F32)
        g16_sb = cpool.tile([P, T * D], BF16)
        off16_sb = cpool.tile([P, NCH], BF16)
        iota16_sb = cpool.tile([P, P], BF16)
        zT_sb = cpool.tile([P, T * P], F32)
        wp_sb = cpool.tile([P, D], F32)
        swrow_sb = cpool.tile([1, D], F32)

        nc.sync.dma_start(
            z_sb[:].rearrange("p (t f) -> p t f", f=D),
            z0_d.ap().rearrange("(t p) f -> p t f", p=P))
        nc.sync.dma_start(w_sb[:], w_d.ap())
        nc.sync.dma_start(brow_sb[:], brow_d.ap())
        nc.sync.dma_start(gbt_sb[:], gbt_d.ap())
        nc.sync.dma_start(deg_sb[:], deg_d.ap())
        nc.sync.dma_start(msk_sb[:], msk_d.ap())
        nc.sync.dma_start(idx_sb[:], idx_d.ap())
        nc.sync.dma_start(off_sb[:], off_d.ap())
        nc.sync.dma_start(cst_sb[:], cst_d.ap())

        nc.vector.memset(ones_sb[:], 1.0)
        nc.vector.memset(onecol_sb[:], 1.0)
        nc.vector.tensor_scalar_mul(negoff_sb[:], off_sb[:], -1.0)
        nc.vector.tensor_copy(off16_sb[:], off_sb[:])
        nc.vector.tensor_copy(iota16_sb[:], iota_sb)
        nc.scalar.sqrt(dis_sb[:], deg_sb[:])
        nc.vector.reciprocal(dis_sb[:], dis_sb[:])

        pag = ctx.enter_context(tc.tile_pool(name="pagg", bufs=2, space="PSUM"))
        pzt = ctx.enter_context(tc.tile_pool(name="pzt", bufs=1, space="PSUM"))
        ph = ctx.enter_context(tc.tile_pool(name="ph", bufs=2, space="PSUM"))
        pst = ctx.enter_context(tc.tile_pool(name="pst", bufs=1, space="PSUM"))

        # replicate per-layer bias rows across partitions (rank-1 matmul)
        for li in range(L):
            bp = pag.tile([P, D], F32, tag="agg")
            nc.tensor.matmul(bp[:], ones_sb[:], brow_sb[:, li * D:(li + 1) * D],
                             start=True, stop=True)
            nc.scalar.copy(brep_sb[:, li * D:(li + 1) * D], bp[:])

        aspool = ctx.enter_context(tc.tile_pool(name="asp", bufs=2))
        a_col = aspool.tile([P, 1], F32, tag="a")
        s_col = aspool.tile([P, 1], F32, tag="s")
        nc.vector.memset(a_col[:], 1.0)
        nc.vector.memset(s_col[:], 0.0)

        ztpool = ctx.enter_context(tc.tile_pool(name="ztp", bufs=3))
        spool = ctx.enter_context(tc.tile_pool(name="sel", bufs=4))
        sqpool = ctx.enter_context(tc.tile_pool(name="sqp", bufs=3))
        msgpool = ctx.enter_context(tc.tile_pool(name="msg", bufs=2))
        smallp = ctx.enter_context(tc.tile_pool(name="small", bufs=2))

        # prologue: transpose the initial z tiles into the persistent zT buffer
        for t in range(T):
            zt_ps = pzt.tile([P, P], F32, tag="zt")
            nc.tensor.transpose(zt_ps[:], z_sb[:, t * D:(t + 1) * D],
                                ident_sb)
            nc.scalar.copy(zT_sb[:, t * P:(t + 1) * P], zt_ps[:])

        nrep = int(os.environ.get("NREP", "1"))
        for li0 in range(L * nrep):
            li = li0 % L
            wl = w_sb[:, li * D:(li + 1) * D]
            brep_l = brep_sb[:, li * D:(li + 1) * D]
            g_fullX = dram.tile([C * TS * P, D], BF16, addr_space="Shared",
                                name=f"g_fx_{li0}")
            g_fullY = dram.tile([C * TSY * P, D], BF16, addr_space="Shared",
                                name=f"g_fy_{li0}")
            st_out = dram.tile([P, 2], F32, addr_space="Shared",
                               name=f"st_out_{li0}")

            def flush_g(lo, hi, g_own_part, g_full_part):
                # convert g[:, lo:hi tiles] to bf16, publish, AllGather.  The
                # X half launches mid-phase-1 so the collective overlaps the
                # remaining tiles' matmuls.
                nc.vector.tensor_copy(g16_sb[:, lo * D:hi * D],
                                      g_sb[:, lo * D:hi * D])
                nc.sync.dma_start(
                    g_own_part[:].rearrange("(t p) f -> p t f", p=P),
                    g16_sb[:, lo * D:hi * D].rearrange("p (t f) -> p t f",
                                                       f=D))
                if "allgather" in ABLATE or "localcomm" in ABLATE:
                    nc.sync.dma_start(g_full_part[0:(hi - lo) * P, :],
                                      g_own_part[:])
                else:
                    nc.gpsimd.collective_compute(
                        "AllGather", ALU.bypass,
                        replica_groups=[list(range(C))],
                        ins=[g_own_part.opt()], outs=[g_full_part.opt()])

            # ---- phase 1: BN folded into weights:
            #   h = bn(z) @ W = z @ (a ⊙ W) + (s @ W);  g = h * dis ----
            nc.scalar.mul(wp_sb[:], wl, a_col[:])
            sw_ps = pst.tile([1, D], F32, tag="sw")
            nc.tensor.matmul(sw_ps[:], s_col[:], wl, start=True, stop=True)
            nc.scalar.copy(swrow_sb[:], sw_ps[:])
            for t in range(T):
                hp = ph.tile([P, D], F32, tag="h")
                nc.tensor.matmul(hp[:], zT_sb[:, t * P:(t + 1) * P], wp_sb[:],
                                 start=True, stop=False)
                nc.tensor.matmul(hp[:], ones_sb[:], swrow_sb[:],
                                 start=False, stop=True)
                nc.scalar.mul(g_sb[:, t * D:(t + 1) * D], hp[:],
                              dis_sb[:, t:t + 1])
                if t == TS - 1:
                    flush_g(0, TS, g_ownX, g_fullX)
            flush_g(TS, T, g_ownY, g_fullY)

            sum_ps = pst.tile([P, 1], F32, tag="sum")
            ssq_ps = pst.tile([P, 1], F32, tag="ssq")

            # ---- phase 2: gather + segment-sum + pointwise + stats ----
            for gi, grp in enumerate(groups):
                acs, acw, bcs, bcw, nA, nB = idx_base[gi]
                msg = msgpool.tile([P, MAXSLOTS, D], BF16, tag="msg")
                if "gather" in ABLATE or "agg" in ABLATE:
                    nc.vector.memset(msg[:, 0:1, :], 0.0)
                else:
                    if nA:
                        nc.gpsimd.dma_gather(
                            msg[:, 0:nA, :], g_fullX[0:C * TS * P, :],
                            idx_sb[:, acs:acs + acw], nA * P, nA * P, D,
                            single_packet=False, queue_num=gi % NSWQ)
                    if nB:
                        nc.gpsimd.dma_gather(
                            msg[:, nA:nA + nB, :], g_fullY[0:C * TSY * P, :],
                            idx_sb[:, bcs:bcs + bcw], nB * P, nB * P, D,
                            single_packet=False, queue_num=(gi + 1) % NSWQ)
                sa = 0
                sb_ = nA
                for t in grp:
                    nch = int(CA[t] + CB[t])
                    if "agg" in ABLATE:
                        nch = 0
                    agg = pag.tile([P, D], F32, tag="agg")
                    for k in range(nch):
                        if k < CA[t]:
                            cc = int(colA[t] + k)
                            slot = sa + k
                        else:
                            cc = int(colB[t] + (k - CA[t]))
                            slot = sb_ + (k - CA[t])
                        sel = spool.tile([P, P], BF16, tag="S")
                        if SMOD and k % SMOD == SMOD - 1:
                            # ACT path: S = relu(1 - (iota - off)^2)
                            nc.scalar.activation(
                                sel[:], iota_sb,
                                mybir.ActivationFunctionType.Square,
                                bias=negoff_sb[:, cc:cc + 1])
                            nc.scalar.activation(
                                sel[:], sel[:],
                                mybir.ActivationFunctionType.Relu,
                                bias=onecol_sb[:], scale=-1.0)
                        else:
                            nc.vector.tensor_single_scalar(
                                sel[:], iota16_sb[:], off_sb[:, cc:cc + 1],
                                ALU.is_equal)
                        nc.tensor.matmul(agg[:], sel[:], msg[:, slot, :],
                                         start=(k == 0), stop=(k == nch - 1))
                    sa += int(CA[t])
                    sb_ += int(CB[t])

                    zsl = z_sb[:, t * D:(t + 1) * D]
                    gsl = g_sb[:, t * D:(t + 1) * D]
                    if nch:
                        nc.vector.tensor_add(zsl, agg[:], gsl)
                        nc.vector.scalar_tensor_tensor(
                            zsl, zsl, dis_sb[:, t:t + 1], brep_l,
                            op0=ALU.mult, op1=ALU.add)
                    else:
                        nc.vector.scalar_tensor_tensor(
                            zsl, gsl, dis_sb[:, t:t + 1], brep_l,
                            op0=ALU.mult, op1=ALU.add)
                    nc.scalar.activation(zsl, zsl,
                                         mybir.ActivationFunctionType.Relu)
                    zt_ps = pzt.tile([P, P], F32, tag="zt")
                    nc.tensor.transpose(zt_ps[:], zsl, ident_sb)
                    nc.scalar.copy(zT_sb[:, t * P:(t + 1) * P], zt_ps[:])
                    sq = sqpool.tile([P, D], F32, tag="sq")
                    nc.scalar.square(sq[:], zsl)
                    if "stats" not in ABLATE:
                        nc.tensor.matmul(sum_ps[:], zsl, msk_sb[:, t:t + 1],
                                         start=(t == 0), stop=(t == T - 1),
                                         skip_group_check=True)
                        nc.tensor.matmul(ssq_ps[:], sq[:], msk_sb[:, t:t + 1],
                                         start=(t == 0), stop=(t == T - 1),
                                         skip_group_check=True)

            # ---- phase 3: BN stats AllReduce + a/s columns ----
            if "stats" in ABLATE:
                continue
            st_sb = smallp.tile([P, 2], F32, tag="st")
            nc.vector.tensor_copy(st_sb[:, 0:1], sum_ps[:])
            nc.vector.tensor_copy(st_sb[:, 1:2], ssq_ps[:])
            nc.sync.dma_start(st_in[:], st_sb[:])
            if "localcomm" in ABLATE:
                nc.sync.dma_start(st_out[0:P, :], st_in[:])
            else:
                nc.gpsimd.collective_compute(
                    "AllReduce", ALU.add,
                    replica_groups=[list(range(C))],
                    ins=[st_in.opt()], outs=[st_out.opt()])
            st2 = smallp.tile([P, 2], F32, tag="st2")
            nc.sync.dma_start(st2[:], st_out[:])
            mean = smallp.tile([P, 1], F32, tag="mean")
            ex2 = smallp.tile([P, 1], F32, tag="ex2")
            m2 = smallp.tile([P, 1], F32, tag="m2")
            var = smallp.tile([P, 1], F32, tag="var")
            sd = smallp.tile([P, 1], F32, tag="sd")
            isd = smallp.tile([P, 1], F32, tag="isd")
            tmp = smallp.tile([P, 1], F32, tag="tmp")
            nc.vector.tensor_scalar_mul(mean[:], st2[:, 0:1], 1.0 / NN)
            nc.vector.tensor_scalar_mul(ex2[:], st2[:, 1:2], 1.0 / NN)
            nc.scalar.square(m2[:], mean[:])
            nc.vector.tensor_sub(var[:], ex2[:], m2[:])
            nc.scalar.activation(sd[:], var[:],
                                 mybir.ActivationFunctionType.Sqrt,
                                 bias=eps_sb)
            nc.vector.reciprocal(isd[:], sd[:])
            a_col = aspool.tile([P, 1], F32, tag="a")
            s_col = aspool.tile([P, 1], F32, tag="s")
            nc.vector.tensor_mul(a_col[:], gbt_sb[:, li:li + 1], isd[:])
            nc.vector.tensor_mul(tmp[:], mean[:], a_col[:])
            nc.vector.tensor_sub(s_col[:], gbt_sb[:, L + li:L + li + 1], tmp[:])

        # ---- final BN apply (transposed) + output ----
        for t in range(T):
            zo_sb = ztpool.tile([P, P], F32, tag="zt")
            nc.vector.tensor_scalar(zo_sb[:], zT_sb[:, t * P:(t + 1) * P],
                                    a_col[:], s_col[:], ALU.mult, ALU.add)
            nc.sync.dma_start(zo_d[t * P:(t + 1) * P, :], zo_sb[:])


# --------------------------------------------------------------------------
# Entry points
# --------------------------------------------------------------------------

def make_in_maps(meta, per_core, Ws, bs, gammas, betas):
    Ws = np.asarray(Ws, np.float32)
    wimg = np.concatenate([Ws[li] for li in range(L)], axis=1)       # [P, L*D]
    brow = np.concatenate([np.asarray(bs[li], np.float32)
                           for li in range(L)])[None, :]             # [1, L*D]
    gbt = np.stack([np.asarray(gammas[li], np.float32) for li in range(L)]
                   + [np.asarray(betas[li], np.float32) for li in range(L)],
                   axis=1)                                           # [P, 2*L]
    consts = np.zeros((P, 2 * P + 1), np.float32)
    consts[:, 0:P] = np.arange(P, dtype=np.float32)[None, :]   # iota row
    consts[:, P:2 * P] = np.eye(P, dtype=np.float32)           # identity
    consts[:, 2 * P] = EPS
    in_maps = []
    for ci in range(C):
        pc = per_core[ci]
        in_maps.append(dict(
            z0=pc["z0"], wimg=wimg, brow=brow, gbt=gbt,
            degT=pc["degT"], mskT=pc["mskT"],
            idximg=pc["idximg"], offimg=pc["offimg"], consts=consts))
    return in_maps


def unshard(meta, outs):
    """outs: list of 8 per-core zout arrays [T*P, P] (transposed tiles)."""
    T = meta["T"]
    flat = np.stack([o.reshape(T, P, P).transpose(0, 2, 1).reshape(T * P, P)
                     for o in outs])                    # [C, R, D] row-major
    flat = flat.reshape(C * T * P, D)
    return flat[meta["pad_id"]]


def build_nc(meta):
    nc = bacc.Bacc("TRN2", target_bir_lowering=False, debug=False,
                   num_devices=C, num_swdge_queues=NSWQ)
    build(nc, meta)
    nc.compile()
    return nc


class Runner:
    """Cached PJRT executable for the SPMD bass program (mirrors
    bass2jax.run_bass_via_pjrt's multi-core branch, but reusable so repeated
    executions don't re-trace/compile)."""

    def __init__(self, nc):
        import jax
        from jax.experimental.shard_map import shard_map
        from jax.sharding import Mesh, PartitionSpec
        from concourse import bass2jax as b2j

        b2j.install_neuronx_cc_hook()
        self.nc = nc
        partition_name = (nc.partition_id_tensor.name
                          if nc.partition_id_tensor else None)
        in_names, out_names, out_avals, zero_shapes = [], [], [], []
        for alloc in nc.m.functions[0].allocations:
            if not isinstance(alloc, mybir.MemoryLocationSet):
                continue
            name = alloc.memorylocations[0].name
            if alloc.kind == "ExternalInput":
                if name != partition_name:
                    in_names.append(name)
            elif alloc.kind == "ExternalOutput":
                shape = tuple(alloc.tensor_shape)
                dtype = mybir.dt.np(alloc.dtype)
                out_names.append(name)
                out_avals.append(jax.core.ShapedArray(shape, dtype))
                zero_shapes.append((shape, dtype))
        self.in_names = list(in_names)
        self.out_names = out_names
        self.out_avals = out_avals
        self.zero_shapes = zero_shapes
        n_params = len(in_names)
        n_outs = len(out_names)
        all_in_names = in_names + out_names
        if partition_name is not None:
            all_in_names.append(partition_name)

        def _body(*args):
            operands = list(args)
            if partition_name is not None:
                operands.append(b2j.partition_id_tensor())
            outs = b2j._bass_exec_p.bind(
                *operands,
                out_avals=tuple(out_avals),
                in_names=tuple(all_in_names),
                out_names=tuple(out_names),
                lowering_input_output_aliases=(),
                sim_require_finite=True,
                sim_require_nnan=True,
                nc=nc,
            )
            return tuple(outs)

        devices = jax.devices()[:C]
        mesh = Mesh(np.asarray(devices), ("core",))
        in_specs = (PartitionSpec("core"),) * (n_params + n_outs)
        out_specs = (PartitionSpec("core"),) * n_outs
        self.sharded = jax.jit(
            shard_map(_body, mesh=mesh, in_specs=in_specs,
                      out_specs=out_specs, check_rep=False),
            donate_argnums=tuple(range(n_params, n_params + n_outs)),
            keep_unused=True,
        )
        self._jax = jax
        self._sharding = jax.sharding.NamedSharding(mesh, PartitionSpec("core"))

    def put_inputs(self, concat_in):
        return [self._jax.device_put(a, self._sharding) for a in concat_in]

    def put_zeros(self):
        return [self._jax.device_put(np.zeros((C * s[0], *s[1:]), d),
                                     self._sharding)
                for s, d in self.zero_shapes]

    def pack(self, in_maps):
        return [np.concatenate([np.asarray(m[n]) for m in in_maps], axis=0)
                for n in self.in_names]

    def run_packed(self, concat_in):
        zeros = [np.zeros((C * s[0], *s[1:]), d) for s, d in self.zero_shapes]
        out_arrs = self.sharded(*concat_in, *zeros)
        self._jax.block_until_ready(out_arrs)
        return out_arrs

    def run(self, in_maps):
        out_arrs = self.run_packed(self.pack(in_maps))
        return [
            {n: np.asarray(out_arrs[i]).reshape(C, *self.out_avals[i].shape)[c]
             for i, n in enumerate(self.out_names)}
            for c in range(C)
        ]


def kernel(x=None, edge_index=None, Ws=None, bs=None, gammas=None, betas=None):
    x = np.asarray(x, np.float32)
    meta, per_core = prep(x, np.asarray(edge_index))
    in_maps = make_in_maps(meta, per_core, Ws, bs, gammas, betas)
    nc = build_nc(meta)
    results = Runner(nc).run(in_maps)
    outs = [r["zout"] for r in results]
    return unshard(meta, outs).astype(np.float32)



# revision 25
# speedup vs baseline: 1.0871x; 1.0871x over previous
"""GCN encoder (4x GCNConv+ReLU+BatchNorm) as a Trainium2 Bass kernel on 8 cores.

Sharding: destination nodes are partitioned into 8*T degree-balanced buckets of
128 rows (one bucket = one 128-row "tile" on one core).  Per layer, each core:
  1. PE-transposes each z tile (fused with the BatchNorm apply of the previous
     layer), computes h = z @ W on PE, scales rows by dis = 1/sqrt(deg) -> g.
  2. AllGathers g into a full [NPAD, 128] DRAM table.
  3. For each dst tile, gathers the source rows of its (host-bucketed, padded)
     edges with SWDGE dma_gather, builds one-hot selector matrices on DVE
     (is_equal against an iota row) and segment-sums messages into PSUM with
     PE matmuls (out = S.T @ msgs).
  4. z' = relu(dis * (agg + g) + b); BN statistics via matmuls with a validity
     mask column; stats AllReduce'd across cores ([128, 2]).
The final BN apply happens in the transposed orientation, so the kernel output
is per-tile transposed; the host undoes the permutation and transposition.
"""

import math
import os
from contextlib import ExitStack

import numpy as np

import concourse.bass as bass
import concourse.bacc as bacc
import concourse.mybir as mybir
import concourse.tile as tile
from concourse import bass_utils

P = 128          # partitions / tile rows
D = 128          # feature dim
L = 4            # layers
C = 8            # cores
HALF = 32768     # int16 gather-index limit
EPS = 1e-5
F32 = mybir.dt.float32
BF16 = mybir.dt.bfloat16
I16 = mybir.dt.int16
ALU = mybir.AluOpType
GROUP_TILES = 4  # dst tiles per gather batch
ABLATE = set()   # debug: subset of {"agg", "gather", "stats", "allgather"}
SMOD = 0         # every SMOD-th selector build goes to ACT (0 = never; ACT latency makes this a loss)
NSWQ = 2         # SWDGE queues; gathers alternate queues to overlap desc-gen/transfer


# --------------------------------------------------------------------------
# Host-side sharding
# --------------------------------------------------------------------------

def _balance_buckets(deg, n_buckets):
    """Snake round-robin over degree-sorted nodes -> (bucket, slot) per node.
    Each bucket gets <= ceil(N / n_buckets) nodes with near-equal degree sums."""
    n = deg.shape[0]
    order = np.argsort(-deg, kind="stable")
    idx = np.arange(n)
    rounds = idx // n_buckets
    pos = idx % n_buckets
    b_of = np.where(rounds % 2 == 0, pos, n_buckets - 1 - pos)
    bucket = np.empty(n, np.int64)
    slot = np.empty(n, np.int64)
    bucket[order] = b_of
    slot[order] = rounds
    return bucket, slot


def _wrap_idx(v):
    """Wrap a flat int array into the SWDGE index image rows: img[p, s] =
    v[s*16 + p % 16], replicated across the 8 groups of 16 partitions."""
    n = v.shape[0]
    assert n % 16 == 0
    blk = v.reshape(n // 16, 16).T.astype(np.int16)   # [16, n/16]
    return np.tile(blk, (8, 1))                       # [128, n/16]


def prep(x, edge_index):
    """Shard the graph. Returns (meta, per_core_inputs: list of dicts)."""
    n_nodes, d = x.shape
    assert d == D
    src = np.asarray(edge_index[0], dtype=np.int64)
    dst = np.asarray(edge_index[1], dtype=np.int64)

    T = -(-n_nodes // (C * P))          # tiles per core
    R = T * P                           # padded rows per core
    NPAD = C * R
    assert NPAD - HALF < HALF, "second half-table must also fit int16 indices"

    deg = np.bincount(dst, minlength=n_nodes)
    bucket, slot = _balance_buckets(deg, C * T)
    assert slot.max() < P
    core_of = bucket // T
    tile_of = bucket % T
    pad_id = core_of * R + tile_of * P + slot        # padded global row id

    cnt = np.bincount(bucket, minlength=C * T).reshape(C, T)

    TS = -(-T // 2)                     # tiles per core in table X
    TSY = T - TS                        # tiles per core in table Y
    assert C * TS * P <= 32768 and C * TSY * P <= 32768

    e_core = core_of[dst]
    e_tile = tile_of[dst]
    e_off = slot[dst]
    src_core = core_of[src]
    src_tile = tile_of[src]
    src_slot = slot[src]
    e_half = (src_tile >= TS).astype(np.int64)    # 0 -> table X, 1 -> table Y
    e_srcp = np.where(e_half == 0,
                      src_core * (TS * P) + src_tile * P + src_slot,
                      src_core * (TSY * P) + (src_tile - TS) * P + src_slot)

    # chunk budgets per (tile, half): max over cores
    key = ((e_core * T) + e_tile) * 2 + e_half
    ecnt = np.bincount(key, minlength=C * T * 2).reshape(C, T, 2)
    CA = -(-ecnt[:, :, 0].max(axis=0) // P)          # [T]
    CB = -(-ecnt[:, :, 1].max(axis=0) // P)

    groups = [list(range(s, min(s + GROUP_TILES, T)))
              for s in range(0, T, GROUP_TILES)]

    # chunk column bases (t-major, A chunks then B chunks per tile)
    colA = np.zeros(T, np.int64)
    colB = np.zeros(T, np.int64)
    c = 0
    for t in range(T):
        colA[t] = c
        c += CA[t]
        colB[t] = c
        c += CB[t]
    NCH = int(c)

    # idx image column layout per group: A segment then B segment
    idx_base = []
    ic = 0
    for grp in groups:
        nA = int(sum(CA[t] for t in grp))
        nB = int(sum(CB[t] for t in grp))
        idx_base.append((ic, nA * 8, ic + nA * 8, nB * 8, nA, nB))
        ic += (nA + nB) * 8
    IDXC = ic
    MAXSLOTS = max(a[4] + a[5] for a in idx_base)

    meta = dict(N=n_nodes, T=T, R=R, NPAD=NPAD, CA=CA, CB=CB,
                groups=groups, colA=colA, colB=colB, NCH=NCH,
                idx_base=idx_base, IDXC=IDXC, MAXSLOTS=MAXSLOTS,
                pad_id=pad_id, TS=TS, TSY=TSY)

    # ---------------- per-core images ----------------
    per_core = []
    eorder = np.lexsort((e_half, e_tile, e_core))    # sort edges
    s_core = e_core[eorder]
    s_tile = e_tile[eorder]
    s_half = e_half[eorder]
    s_srcp = e_srcp[eorder]
    s_off = e_off[eorder]

    # start offset of each (core, tile, half) run in the sorted arrays
    runkey = ((s_core * T) + s_tile) * 2 + s_half
    starts = np.searchsorted(runkey, np.arange(C * T * 2))

    for ci in range(C):
        z0 = np.zeros((R, D), np.float32)
        nodes_c = np.where(core_of == ci)[0]
        z0[tile_of[nodes_c] * P + slot[nodes_c]] = x[nodes_c]

        degT = np.ones((P, T), np.float32)
        mskT = np.zeros((P, T), np.float32)
        tt = tile_of[nodes_c]
        ss = slot[nodes_c]
        degT[ss, tt] = deg[nodes_c] + 1.0
        mskT[ss, tt] = 1.0

        offimg = np.full((P, NCH), -1.0, np.float32)
        idx_img = np.zeros((P, IDXC), np.int16)

        for gi, grp in enumerate(groups):
            for hi in (0, 1):
                vs = []
                for t in grp:
                    budget = (CA[t] if hi == 0 else CB[t]) * P
                    if budget == 0:
                        continue
                    k0 = ((ci * T) + t) * 2 + hi
                    a = starts[k0]
                    b = starts[k0 + 1] if k0 + 1 < C * T * 2 else len(runkey)
                    srcs = s_srcp[a:b]
                    offs = s_off[a:b]
                    assert len(srcs) <= budget
                    v = np.zeros(budget, np.int64)
                    v[:len(srcs)] = srcs
                    o = np.full(budget, -1.0, np.float32)
                    o[:len(srcs)] = offs
                    vs.append(v)
                    cb = (colA[t] if hi == 0 else colB[t])
                    nch_t = budget // P
                    offimg[:, cb:cb + nch_t] = o.reshape(nch_t, P).T
                if vs:
                    vflat = np.concatenate(vs)
                    cs = idx_base[gi][0] if hi == 0 else idx_base[gi][2]
                    cw = idx_base[gi][1] if hi == 0 else idx_base[gi][3]
                    assert vflat.shape[0] // 16 == cw  # cols match
                    idx_img[:, cs:cs + cw] = _wrap_idx(vflat)

        per_core.append(dict(z0=z0, degT=degT, mskT=mskT,
                             idximg=idx_img, offimg=offimg))
    return meta, per_core


# --------------------------------------------------------------------------
# Device program
# --------------------------------------------------------------------------

def build(nc, meta):
    T = meta["T"]
    NPAD = meta["NPAD"]
    NN = meta["N"]
    CA, CB = meta["CA"], meta["CB"]
    colA, colB = meta["colA"], meta["colB"]
    groups = meta["groups"]
    idx_base = meta["idx_base"]
    IDXC, NCH, MAXSLOTS = meta["IDXC"], meta["NCH"], meta["MAXSLOTS"]

    z0_d = nc.dram_tensor("z0", [T * P, D], F32, kind="ExternalInput")
    w_d = nc.dram_tensor("wimg", [P, L * D], F32, kind="ExternalInput")
    brow_d = nc.dram_tensor("brow", [1, L * D], F32, kind="ExternalInput")
    gbt_d = nc.dram_tensor("gbt", [P, 2 * L], F32, kind="ExternalInput")
    deg_d = nc.dram_tensor("degT", [P, T], F32, kind="ExternalInput")
    msk_d = nc.dram_tensor("mskT", [P, T], F32, kind="ExternalInput")
    idx_d = nc.dram_tensor("idximg", [P, IDXC], I16, kind="ExternalInput")
    off_d = nc.dram_tensor("offimg", [P, NCH], F32, kind="ExternalInput")
    cst_d = nc.dram_tensor("consts", [P, 2 * P + 1], F32, kind="ExternalInput")
    zo_d = nc.dram_tensor("zout", [T * P, P], F32, kind="ExternalOutput")

    TS, TSY = meta["TS"], meta["TSY"]

    with tile.TileContext(nc) as tc, ExitStack() as ctx:
        dram = ctx.enter_context(tc.tile_pool(name="dram", bufs=1, space="DRAM"))
        g_ownX = dram.tile([TS * P, D], BF16)
        g_ownY = dram.tile([TSY * P, D], BF16)
        st_in = dram.tile([P, 2], F32)

        cpool = ctx.enter_context(tc.tile_pool(name="const", bufs=1))
        z_sb = cpool.tile([P, T * D], F32)
        g_sb = cpool.tile([P, T * D], F32)
        w_sb = cpool.tile([P, L * D], F32)
        brow_sb = cpool.tile([1, L * D], F32)
        brep_sb = cpool.tile([P, L * D], F32)
        gbt_sb = cpool.tile([P, 2 * L], F32)
        deg_sb = cpool.tile([P, T], F32)
        msk_sb = cpool.tile([P, T], F32)
        dis_sb = cpool.tile([P, T], F32)
        idx_sb = cpool.tile([P, IDXC], I16)
        off_sb = cpool.tile([P, NCH], F32)
        cst_sb = cpool.tile([P, 2 * P + 1], F32)
        iota_sb = cst_sb[:, 0:P]
        ident_sb = cst_sb[:, P:2 * P]
        eps_sb = cst_sb[:, 2 * P:2 * P + 1]
        ones_sb = cpool.tile([1, P], F32)
        negoff_sb = cpool.tile([P, NCH], F32)
        onecol_sb = cpool.tile([P, 1], F32)
        g16_sb = cpool.tile([P, T * D], BF16)
        off16_sb = cpool.tile([P, NCH], BF16)
        iota16_sb = cpool.tile([P, P], BF16)
        zT_sb = cpool.tile([P, T * P], F32)
        wp_sb = cpool.tile([P, D], F32)
        swrow_sb = cpool.tile([1, D], F32)

        nc.sync.dma_start(
            z_sb[:].rearrange("p (t f) -> p t f", f=D),
            z0_d.ap().rearrange("(t p) f -> p t f", p=P))
        nc.sync.dma_start(w_sb[:], w_d.ap())
        nc.sync.dma_start(brow_sb[:], brow_d.ap())
        nc.sync.dma_start(gbt_sb[:], gbt_d.ap())
        nc.sync.dma_start(deg_sb[:], deg_d.ap())
        nc.sync.dma_start(msk_sb[:], msk_d.ap())
        nc.sync.dma_start(idx_sb[:], idx_d.ap())
        nc.sync.dma_start(off_sb[:], off_d.ap())
        nc.sync.dma_start(cst_sb[:], cst_d.ap())

        nc.vector.memset(ones_sb[:], 1.0)
        nc.vector.memset(onecol_sb[:], 1.0)
        nc.vector.tensor_scalar_mul(negoff_sb[:], off_sb[:], -1.0)
        nc.vector.tensor_copy(off16_sb[:], off_sb[:])
        nc.vector.tensor_copy(iota16_sb[:], iota_sb)
        nc.scalar.sqrt(dis_sb[:], deg_sb[:])
        nc.vector.reciprocal(dis_sb[:], dis_sb[:])

        pag = ctx.enter_context(tc.tile_pool(name="pagg", bufs=2, space="PSUM"))
        pzt = ctx.enter_context(tc.tile_pool(name="pzt", bufs=1, space="PSUM"))
        ph = ctx.enter_context(tc.tile_pool(name="ph", bufs=2, space="PSUM"))
        pst = ctx.enter_context(tc.tile_pool(name="pst", bufs=1, space="PSUM"))

        # replicate per-layer bias rows across partitions (rank-1 matmul)
        for li in range(L):
            bp = pag.tile([P, D], F32, tag="agg")
            nc.tensor.matmul(bp[:], ones_sb[:], brow_sb[:, li * D:(li + 1) * D],
                             start=True, stop=True)
            nc.scalar.copy(brep_sb[:, li * D:(li + 1) * D], bp[:])

        aspool = ctx.enter_context(tc.tile_pool(name="asp", bufs=2))
        a_col = aspool.tile([P, 1], F32, tag="a")
        s_col = aspool.tile([P, 1], F32, tag="s")
        nc.vector.memset(a_col[:], 1.0)
        nc.vector.memset(s_col[:], 0.0)

        ztpool = ctx.enter_context(tc.tile_pool(name="ztp", bufs=3))
        spool = ctx.enter_context(tc.tile_pool(name="sel", bufs=4))
        sqpool = ctx.enter_context(tc.tile_pool(name="sqp", bufs=3))
        msgpool = ctx.enter_context(tc.tile_pool(name="msg", bufs=3))
        smallp = ctx.enter_context(tc.tile_pool(name="small", bufs=2))

        # prologue: transpose the initial z tiles into the persistent zT buffer
        for t in range(T):
            zt_ps = pzt.tile([P, P], F32, tag="zt")
            nc.tensor.transpose(zt_ps[:], z_sb[:, t * D:(t + 1) * D],
                                ident_sb)
            nc.scalar.copy(zT_sb[:, t * P:(t + 1) * P], zt_ps[:])

        nrep = int(os.environ.get("NREP", "1"))
        for li0 in range(L * nrep):
            li = li0 % L
            wl = w_sb[:, li * D:(li + 1) * D]
            brep_l = brep_sb[:, li * D:(li + 1) * D]
            g_fullX = dram.tile([C * TS * P, D], BF16, addr_space="Shared",
                                name=f"g_fx_{li0}")
            g_fullY = dram.tile([C * TSY * P, D], BF16, addr_space="Shared",
                                name=f"g_fy_{li0}")
            st_out = dram.tile([P, 2], F32, addr_space="Shared",
                               name=f"st_out_{li0}")

            def flush_g(lo, hi, g_own_part, g_full_part):
                # convert g[:, lo:hi tiles] to bf16, publish, AllGather.  The
                # X half launches mid-phase-1 so the collective overlaps the
                # remaining tiles' matmuls.
                nc.vector.tensor_copy(g16_sb[:, lo * D:hi * D],
                                      g_sb[:, lo * D:hi * D])
                nc.sync.dma_start(
                    g_own_part[:].rearrange("(t p) f -> p t f", p=P),
                    g16_sb[:, lo * D:hi * D].rearrange("p (t f) -> p t f",
                                                       f=D))
                if "allgather" in ABLATE or "localcomm" in ABLATE:
                    nc.sync.dma_start(g_full_part[0:(hi - lo) * P, :],
                                      g_own_part[:])
                else:
                    nc.gpsimd.collective_compute(
                        "AllGather", ALU.bypass,
                        replica_groups=[list(range(C))],
                        ins=[g_own_part.opt()], outs=[g_full_part.opt()])

            # ---- phase 1: BN folded into weights:
            #   h = bn(z) @ W = z @ (a ⊙ W) + (s @ W);  g = h * dis ----
            nc.scalar.mul(wp_sb[:], wl, a_col[:])
            sw_ps = pst.tile([1, D], F32, tag="sw")
            nc.tensor.matmul(sw_ps[:], s_col[:], wl, start=True, stop=True)
            nc.scalar.copy(swrow_sb[:], sw_ps[:])
            for t in range(T):
                hp = ph.tile([P, D], F32, tag="h")
                nc.tensor.matmul(hp[:], zT_sb[:, t * P:(t + 1) * P], wp_sb[:],
                                 start=True, stop=False)
                nc.tensor.matmul(hp[:], ones_sb[:], swrow_sb[:],
                                 start=False, stop=True)
                nc.scalar.mul(g_sb[:, t * D:(t + 1) * D], hp[:],
                              dis_sb[:, t:t + 1])
                if t == TS - 1:
                    flush_g(0, TS, g_ownX, g_fullX)
            flush_g(TS, T, g_ownY, g_fullY)

            sum_ps = pst.tile([P, 1], F32, tag="sum")
            ssq_ps = pst.tile([P, 1], F32, tag="ssq")

            # ---- phase 2: gather + segment-sum + pointwise + stats ----
            for gi, grp in enumerate(groups):
                acs, acw, bcs, bcw, nA, nB = idx_base[gi]
                msg = msgpool.tile([P, MAXSLOTS, D], BF16, tag="msg")
                if "gather" in ABLATE or "agg" in ABLATE:
                    nc.vector.memset(msg[:, 0:1, :], 0.0)
                else:
                    if nA:
                        nc.gpsimd.dma_gather(
                            msg[:, 0:nA, :], g_fullX[0:C * TS * P, :],
                            idx_sb[:, acs:acs + acw], nA * P, nA * P, D,
                            single_packet=False, queue_num=gi % NSWQ)
                    if nB:
                        nc.gpsimd.dma_gather(
                            msg[:, nA:nA + nB, :], g_fullY[0:C * TSY * P, :],
                            idx_sb[:, bcs:bcs + bcw], nB * P, nB * P, D,
                            single_packet=False, queue_num=(gi + 1) % NSWQ)
                sa = 0
                sb_ = nA
                for t in grp:
                    nch = int(CA[t] + CB[t])
                    if "agg" in ABLATE:
                        nch = 0
                    agg = pag.tile([P, D], F32, tag="agg")
                    for k in range(nch):
                        if k < CA[t]:
                            cc = int(colA[t] + k)
                            slot = sa + k
                        else:
                            cc = int(colB[t] + (k - CA[t]))
                            slot = sb_ + (k - CA[t])
                        sel = spool.tile([P, P], BF16, tag="S")
                        if SMOD and k % SMOD == SMOD - 1:
                            # ACT path: S = relu(1 - (iota - off)^2)
                            nc.scalar.activation(
                                sel[:], iota_sb,
                                mybir.ActivationFunctionType.Square,
                                bias=negoff_sb[:, cc:cc + 1])
                            nc.scalar.activation(
                                sel[:], sel[:],
                                mybir.ActivationFunctionType.Relu,
                                bias=onecol_sb[:], scale=-1.0)
                        else:
                            nc.vector.tensor_single_scalar(
                                sel[:], iota16_sb[:], off_sb[:, cc:cc + 1],
                                ALU.is_equal)
                        nc.tensor.matmul(agg[:], sel[:], msg[:, slot, :],
                                         start=(k == 0), stop=(k == nch - 1))
                    sa += int(CA[t])
                    sb_ += int(CB[t])

                    zsl = z_sb[:, t * D:(t + 1) * D]
                    gsl = g_sb[:, t * D:(t + 1) * D]
                    if nch:
                        nc.vector.tensor_add(zsl, agg[:], gsl)
                        nc.vector.scalar_tensor_tensor(
                            zsl, zsl, dis_sb[:, t:t + 1], brep_l,
                            op0=ALU.mult, op1=ALU.add)
                    else:
                        nc.vector.scalar_tensor_tensor(
                            zsl, gsl, dis_sb[:, t:t + 1], brep_l,
                            op0=ALU.mult, op1=ALU.add)
                    nc.scalar.activation(zsl, zsl,
                                         mybir.ActivationFunctionType.Relu)
                    zt_ps = pzt.tile([P, P], F32, tag="zt")
                    nc.tensor.transpose(zt_ps[:], zsl, ident_sb)
                    nc.scalar.copy(zT_sb[:, t * P:(t + 1) * P], zt_ps[:])
                    sq = sqpool.tile([P, D], F32, tag="sq")
                    nc.scalar.square(sq[:], zsl)
                    if "stats" not in ABLATE:
                        nc.tensor.matmul(sum_ps[:], zsl, msk_sb[:, t:t + 1],
                                         start=(t == 0), stop=(t == T - 1),
                                         skip_group_check=True)
                        nc.tensor.matmul(ssq_ps[:], sq[:], msk_sb[:, t:t + 1],
                                         start=(t == 0), stop=(t == T - 1),
                                         skip_group_check=True)

            # ---- phase 3: BN stats AllReduce + a/s columns ----
            if "stats" in ABLATE:
                continue
            st_sb = smallp.tile([P, 2], F32, tag="st")
            nc.vector.tensor_copy(st_sb[:, 0:1], sum_ps[:])
            nc.vector.tensor_copy(st_sb[:, 1:2], ssq_ps[:])
            nc.sync.dma_start(st_in[:], st_sb[:])
            if "localcomm" in ABLATE:
                nc.sync.dma_start(st_out[0:P, :], st_in[:])
            else:
                nc.gpsimd.collective_compute(
                    "AllReduce", ALU.add,
                    replica_groups=[list(range(C))],
                    ins=[st_in.opt()], outs=[st_out.opt()])
            st2 = smallp.tile([P, 2], F32, tag="st2")
            nc.sync.dma_start(st2[:], st_out[:])
            mean = smallp.tile([P, 1], F32, tag="mean")
            ex2 = smallp.tile([P, 1], F32, tag="ex2")
            m2 = smallp.tile([P, 1], F32, tag="m2")
            var = smallp.tile([P, 1], F32, tag="var")
            sd = smallp.tile([P, 1], F32, tag="sd")
            isd = smallp.tile([P, 1], F32, tag="isd")
            tmp = smallp.tile([P, 1], F32, tag="tmp")
            nc.vector.tensor_scalar_mul(mean[:], st2[:, 0:1], 1.0 / NN)
            nc.vector.tensor_scalar_mul(ex2[:], st2[:, 1:2], 1.0 / NN)
            nc.scalar.square(m2[:], mean[:])
            nc.vector.tensor_sub(var[:], ex2[:], m2[:])
            nc.scalar.activation(sd[:], var[:],
                                 mybir.ActivationFunctionType.Sqrt,
                                 bias=eps_sb)
            nc.vector.reciprocal(isd[:], sd[:])
            a_col = aspool.tile([P, 1], F32, tag="a")
            s_col = aspool.tile([P, 1], F32, tag="s")
            nc.vector.tensor_mul(a_col[:], gbt_sb[:, li:li + 1], isd[:])
            nc.vector.tensor_mul(tmp[:], mean[:], a_col[:])
            nc.vector.tensor_sub(s_col[:], gbt_sb[:, L + li:L + li + 1], tmp[:])

        # ---- final BN apply (transposed) + output ----
        for t in range(T):
            zo_sb = ztpool.tile([P, P], F32, tag="zt")
            nc.vector.tensor_scalar(zo_sb[:], zT_sb[:, t * P:(t + 1) * P],
                                    a_col[:], s_col[:], ALU.mult, ALU.add)
            nc.sync.dma_start(zo_d[t * P:(t + 1) * P, :], zo_sb[:])


# --------------------------------------------------------------------------
# Entry points
# --------------------------------------------------------------------------

def make_in_maps(meta, per_core, Ws, bs, gammas, betas):
    Ws = np.asarray(Ws, np.float32)
    wimg = np.concatenate([Ws[li] for li in range(L)], axis=1)       # [P, L*D]
    brow = np.concatenate([np.asarray(bs[li], np.float32)
                           for li in range(L)])[None, :]             # [1, L*D]
    gbt = np.stack([np.asarray(gammas[li], np.float32) for li in range(L)]
                   + [np.asarray(betas[li], np.float32) for li in range(L)],
                   axis=1)                                           # [P, 2*L]
    consts = np.zeros((P, 2 * P + 1), np.float32)
    consts[:, 0:P] = np.arange(P, dtype=np.float32)[None, :]   # iota row
    consts[:, P:2 * P] = np.eye(P, dtype=np.float32)           # identity
    consts[:, 2 * P] = EPS
    in_maps = []
    for ci in range(C):
        pc = per_core[ci]
        in_maps.append(dict(
            z0=pc["z0"], wimg=wimg, brow=brow, gbt=gbt,
            degT=pc["degT"], mskT=pc["mskT"],
            idximg=pc["idximg"], offimg=pc["offimg"], consts=consts))
    return in_maps


def unshard(meta, outs):
    """outs: list of 8 per-core zout arrays [T*P, P] (transposed tiles)."""
    T = meta["T"]
    flat = np.stack([o.reshape(T, P, P).transpose(0, 2, 1).reshape(T * P, P)
                     for o in outs])                    # [C, R, D] row-major
    flat = flat.reshape(C * T * P, D)
    return flat[meta["pad_id"]]


def build_nc(meta):
    nc = bacc.Bacc("TRN2", target_bir_lowering=False, debug=False,
                   num_devices=C, num_swdge_queues=NSWQ)
    build(nc, meta)
    nc.compile()
    return nc


class Runner:
    """Cached PJRT executable for the SPMD bass program (mirrors
    bass2jax.run_bass_via_pjrt's multi-core branch, but reusable so repeated
    executions don't re-trace/compile)."""

    def __init__(self, nc):
        import jax
        from jax.experimental.shard_map import shard_map
        from jax.sharding import Mesh, PartitionSpec
        from concourse import bass2jax as b2j

        b2j.install_neuronx_cc_hook()
        self.nc = nc
        partition_name = (nc.partition_id_tensor.name
                          if nc.partition_id_tensor else None)
        in_names, out_names, out_avals, zero_shapes = [], [], [], []
        for alloc in nc.m.functions[0].allocations:
            if not isinstance(alloc, mybir.MemoryLocationSet):
                continue
            name = alloc.memorylocations[0].name
            if alloc.kind == "ExternalInput":
                if name != partition_name:
                    in_names.append(name)
            elif alloc.kind == "ExternalOutput":
                shape = tuple(alloc.tensor_shape)
                dtype = mybir.dt.np(alloc.dtype)
                out_names.append(name)
                out_avals.append(jax.core.ShapedArray(shape, dtype))
                zero_shapes.append((shape, dtype))
        self.in_names = list(in_names)
        self.out_names = out_names
        self.out_avals = out_avals
        self.zero_shapes = zero_shapes
        n_params = len(in_names)
        n_outs = len(out_names)
        all_in_names = in_names + out_names
        if partition_name is not None:
            all_in_names.append(partition_name)

        def _body(*args):
            operands = list(args)
            if partition_name is not None:
                operands.append(b2j.partition_id_tensor())
            outs = b2j._bass_exec_p.bind(
                *operands,
                out_avals=tuple(out_avals),
                in_names=tuple(all_in_names),
                out_names=tuple(out_names),
                lowering_input_output_aliases=(),
                sim_require_finite=True,
                sim_require_nnan=True,
                nc=nc,
            )
            return tuple(outs)

        devices = jax.devices()[:C]
        mesh = Mesh(np.asarray(devices), ("core",))
        in_specs = (PartitionSpec("core"),) * (n_params + n_outs)
        out_specs = (PartitionSpec("core"),) * n_outs
        self.sharded = jax.jit(
            shard_map(_body, mesh=mesh, in_specs=in_specs,
                      out_specs=out_specs, check_rep=False),
            donate_argnums=tuple(range(n_params, n_params + n_outs)),
            keep_unused=True,
        )
        self._jax = jax
        self._sharding = jax.sharding.NamedSharding(mesh, PartitionSpec("core"))

    def put_inputs(self, concat_in):
        return [self._jax.device_put(a, self._sharding) for a in concat_in]

    def put_zeros(self):
        return [self._jax.device_put(np.zeros((C * s[0], *s[1:]), d),
                                     self._sharding)
                for s, d in self.zero_shapes]

    def pack(self, in_maps):
        return [np.concatenate([np.asarray(m[n]) for m in in_maps], axis=0)
                for n in self.in_names]

    def run_packed(self, concat_in):
        zeros = [np.zeros((C * s[0], *s[1:]), d) for s, d in self.zero_shapes]
        out_arrs = self.sharded(*concat_in, *zeros)
        self._jax.block_until_ready(out_arrs)
        return out_arrs

    def run(self, in_maps):
        out_arrs = self.run_packed(self.pack(in_maps))
        return [
            {n: np.asarray(out_arrs[i]).reshape(C, *self.out_avals[i].shape)[c]
             for i, n in enumerate(self.out_names)}
            for c in range(C)
        ]


def kernel(x=None, edge_index=None, Ws=None, bs=None, gammas=None, betas=None):
    x = np.asarray(x, np.float32)
    meta, per_core = prep(x, np.asarray(edge_index))
    in_maps = make_in_maps(meta, per_core, Ws, bs, gammas, betas)
    nc = build_nc(meta)
    results = Runner(nc).run(in_maps)
    outs = [r["zout"] for r in results]
    return unshard(meta, outs).astype(np.float32)



# revision 26
# speedup vs baseline: 1.0990x; 1.0110x over previous
"""GCN encoder (4x GCNConv+ReLU+BatchNorm) as a Trainium2 Bass kernel on 8 cores.

Sharding: destination nodes are partitioned into 8*T degree-balanced buckets of
128 rows (one bucket = one 128-row "tile" on one core).  Per layer, each core:
  1. PE-transposes each z tile (fused with the BatchNorm apply of the previous
     layer), computes h = z @ W on PE, scales rows by dis = 1/sqrt(deg) -> g.
  2. AllGathers g into a full [NPAD, 128] DRAM table.
  3. For each dst tile, gathers the source rows of its (host-bucketed, padded)
     edges with SWDGE dma_gather, builds one-hot selector matrices on DVE
     (is_equal against an iota row) and segment-sums messages into PSUM with
     PE matmuls (out = S.T @ msgs).
  4. z' = relu(dis * (agg + g) + b); BN statistics via matmuls with a validity
     mask column; stats AllReduce'd across cores ([128, 2]).
The final BN apply happens in the transposed orientation, so the kernel output
is per-tile transposed; the host undoes the permutation and transposition.
"""

import math
import os
from contextlib import ExitStack

import numpy as np

import concourse.bass as bass
import concourse.bacc as bacc
import concourse.mybir as mybir
import concourse.tile as tile
from concourse import bass_utils

P = 128          # partitions / tile rows
D = 128          # feature dim
L = 4            # layers
C = 8            # cores
HALF = 32768     # int16 gather-index limit
EPS = 1e-5
F32 = mybir.dt.float32
BF16 = mybir.dt.bfloat16
I16 = mybir.dt.int16
ALU = mybir.AluOpType
GROUP_TILES = 4  # dst tiles per gather batch
ABLATE = set()   # debug: subset of {"agg", "gather", "stats", "allgather"}
SMOD = 0         # every SMOD-th selector build goes to ACT (0 = never; ACT latency makes this a loss)
NSWQ = 2         # SWDGE queues; gathers alternate queues to overlap desc-gen/transfer


# --------------------------------------------------------------------------
# Host-side sharding
# --------------------------------------------------------------------------

def _balance_buckets(deg, n_buckets):
    """Snake round-robin over degree-sorted nodes -> (bucket, slot) per node.
    Each bucket gets <= ceil(N / n_buckets) nodes with near-equal degree sums."""
    n = deg.shape[0]
    order = np.argsort(-deg, kind="stable")
    idx = np.arange(n)
    rounds = idx // n_buckets
    pos = idx % n_buckets
    b_of = np.where(rounds % 2 == 0, pos, n_buckets - 1 - pos)
    bucket = np.empty(n, np.int64)
    slot = np.empty(n, np.int64)
    bucket[order] = b_of
    slot[order] = rounds
    return bucket, slot


def _wrap_idx(v):
    """Wrap a flat int array into the SWDGE index image rows: img[p, s] =
    v[s*16 + p % 16], replicated across the 8 groups of 16 partitions."""
    n = v.shape[0]
    assert n % 16 == 0
    blk = v.reshape(n // 16, 16).T.astype(np.int16)   # [16, n/16]
    return np.tile(blk, (8, 1))                       # [128, n/16]


def prep(x, edge_index):
    """Shard the graph. Returns (meta, per_core_inputs: list of dicts)."""
    n_nodes, d = x.shape
    assert d == D
    src = np.asarray(edge_index[0], dtype=np.int64)
    dst = np.asarray(edge_index[1], dtype=np.int64)

    T = -(-n_nodes // (C * P))          # tiles per core
    R = T * P                           # padded rows per core
    NPAD = C * R
    assert NPAD - HALF < HALF, "second half-table must also fit int16 indices"

    deg = np.bincount(dst, minlength=n_nodes)
    bucket, slot = _balance_buckets(deg, C * T)
    assert slot.max() < P
    core_of = bucket // T
    tile_of = bucket % T
    pad_id = core_of * R + tile_of * P + slot        # padded global row id

    cnt = np.bincount(bucket, minlength=C * T).reshape(C, T)

    TS = -(-T // 2)                     # tiles per core in table X
    TSY = T - TS                        # tiles per core in table Y
    assert C * TS * P <= 32768 and C * TSY * P <= 32768

    e_core = core_of[dst]
    e_tile = tile_of[dst]
    e_off = slot[dst]
    src_core = core_of[src]
    src_tile = tile_of[src]
    src_slot = slot[src]
    e_half = (src_tile >= TS).astype(np.int64)    # 0 -> table X, 1 -> table Y
    e_srcp = np.where(e_half == 0,
                      src_core * (TS * P) + src_tile * P + src_slot,
                      src_core * (TSY * P) + (src_tile - TS) * P + src_slot)

    # chunk budgets per (tile, half): max over cores
    key = ((e_core * T) + e_tile) * 2 + e_half
    ecnt = np.bincount(key, minlength=C * T * 2).reshape(C, T, 2)
    CA = -(-ecnt[:, :, 0].max(axis=0) // P)          # [T]
    CB = -(-ecnt[:, :, 1].max(axis=0) // P)

    groups = [list(range(s, min(s + GROUP_TILES, T)))
              for s in range(0, T, GROUP_TILES)]

    # chunk column bases (t-major, A chunks then B chunks per tile)
    colA = np.zeros(T, np.int64)
    colB = np.zeros(T, np.int64)
    c = 0
    for t in range(T):
        colA[t] = c
        c += CA[t]
        colB[t] = c
        c += CB[t]
    NCH = int(c)

    # idx image column layout per group: A segment then B segment
    idx_base = []
    ic = 0
    for grp in groups:
        nA = int(sum(CA[t] for t in grp))
        nB = int(sum(CB[t] for t in grp))
        idx_base.append((ic, nA * 8, ic + nA * 8, nB * 8, nA, nB))
        ic += (nA + nB) * 8
    IDXC = ic
    MAXSLOTS = max(a[4] + a[5] for a in idx_base)

    meta = dict(N=n_nodes, T=T, R=R, NPAD=NPAD, CA=CA, CB=CB,
                groups=groups, colA=colA, colB=colB, NCH=NCH,
                idx_base=idx_base, IDXC=IDXC, MAXSLOTS=MAXSLOTS,
                pad_id=pad_id, TS=TS, TSY=TSY)

    # ---------------- per-core images ----------------
    per_core = []
    eorder = np.lexsort((e_half, e_tile, e_core))    # sort edges
    s_core = e_core[eorder]
    s_tile = e_tile[eorder]
    s_half = e_half[eorder]
    s_srcp = e_srcp[eorder]
    s_off = e_off[eorder]

    # start offset of each (core, tile, half) run in the sorted arrays
    runkey = ((s_core * T) + s_tile) * 2 + s_half
    starts = np.searchsorted(runkey, np.arange(C * T * 2))

    for ci in range(C):
        z0 = np.zeros((R, D), np.float32)
        nodes_c = np.where(core_of == ci)[0]
        z0[tile_of[nodes_c] * P + slot[nodes_c]] = x[nodes_c]

        degT = np.ones((P, T), np.float32)
        mskT = np.zeros((P, T), np.float32)
        tt = tile_of[nodes_c]
        ss = slot[nodes_c]
        degT[ss, tt] = deg[nodes_c] + 1.0
        mskT[ss, tt] = 1.0

        offimg = np.full((P, NCH), -1.0, np.float32)
        idx_img = np.zeros((P, IDXC), np.int16)

        for gi, grp in enumerate(groups):
            for hi in (0, 1):
                vs = []
                for t in grp:
                    budget = (CA[t] if hi == 0 else CB[t]) * P
                    if budget == 0:
                        continue
                    k0 = ((ci * T) + t) * 2 + hi
                    a = starts[k0]
                    b = starts[k0 + 1] if k0 + 1 < C * T * 2 else len(runkey)
                    srcs = s_srcp[a:b]
                    offs = s_off[a:b]
                    assert len(srcs) <= budget
                    v = np.zeros(budget, np.int64)
                    v[:len(srcs)] = srcs
                    o = np.full(budget, -1.0, np.float32)
                    o[:len(srcs)] = offs
                    vs.append(v)
                    cb = (colA[t] if hi == 0 else colB[t])
                    nch_t = budget // P
                    offimg[:, cb:cb + nch_t] = o.reshape(nch_t, P).T
                if vs:
                    vflat = np.concatenate(vs)
                    cs = idx_base[gi][0] if hi == 0 else idx_base[gi][2]
                    cw = idx_base[gi][1] if hi == 0 else idx_base[gi][3]
                    assert vflat.shape[0] // 16 == cw  # cols match
                    idx_img[:, cs:cs + cw] = _wrap_idx(vflat)

        per_core.append(dict(z0=z0, degT=degT, mskT=mskT,
                             idximg=idx_img, offimg=offimg))
    return meta, per_core


# --------------------------------------------------------------------------
# Device program
# --------------------------------------------------------------------------

def build(nc, meta):
    T = meta["T"]
    NPAD = meta["NPAD"]
    NN = meta["N"]
    CA, CB = meta["CA"], meta["CB"]
    colA, colB = meta["colA"], meta["colB"]
    groups = meta["groups"]
    idx_base = meta["idx_base"]
    IDXC, NCH, MAXSLOTS = meta["IDXC"], meta["NCH"], meta["MAXSLOTS"]

    z0_d = nc.dram_tensor("z0", [T * P, D], F32, kind="ExternalInput")
    w_d = nc.dram_tensor("wimg", [P, L * D], F32, kind="ExternalInput")
    brow_d = nc.dram_tensor("brow", [1, L * D], F32, kind="ExternalInput")
    gbt_d = nc.dram_tensor("gbt", [P, 2 * L], F32, kind="ExternalInput")
    deg_d = nc.dram_tensor("degT", [P, T], F32, kind="ExternalInput")
    msk_d = nc.dram_tensor("mskT", [P, T], F32, kind="ExternalInput")
    idx_d = nc.dram_tensor("idximg", [P, IDXC], I16, kind="ExternalInput")
    off_d = nc.dram_tensor("offimg", [P, NCH], F32, kind="ExternalInput")
    cst_d = nc.dram_tensor("consts", [P, 2 * P + 1], F32, kind="ExternalInput")
    zo_d = nc.dram_tensor("zout", [T * P, P], F32, kind="ExternalOutput")

    TS, TSY = meta["TS"], meta["TSY"]

    with tile.TileContext(nc) as tc, ExitStack() as ctx:
        dram = ctx.enter_context(tc.tile_pool(name="dram", bufs=1, space="DRAM"))
        g_ownX = dram.tile([TS * P, D], BF16)
        g_ownY = dram.tile([TSY * P, D], BF16)
        st_in = dram.tile([P, 2], F32)

        cpool = ctx.enter_context(tc.tile_pool(name="const", bufs=1))
        z_sb = cpool.tile([P, T * D], F32)
        g_sb = cpool.tile([P, T * D], F32)
        w_sb = cpool.tile([P, L * D], F32)
        brow_sb = cpool.tile([1, L * D], F32)
        brep_sb = cpool.tile([P, L * D], F32)
        gbt_sb = cpool.tile([P, 2 * L], F32)
        deg_sb = cpool.tile([P, T], F32)
        msk_sb = cpool.tile([P, T], F32)
        dis_sb = cpool.tile([P, T], F32)
        idx_sb = cpool.tile([P, IDXC], I16)
        off_sb = cpool.tile([P, NCH], F32)
        cst_sb = cpool.tile([P, 2 * P + 1], F32)
        iota_sb = cst_sb[:, 0:P]
        ident_sb = cst_sb[:, P:2 * P]
        eps_sb = cst_sb[:, 2 * P:2 * P + 1]
        ones_sb = cpool.tile([1, P], F32)
        negoff_sb = cpool.tile([P, NCH], F32)
        onecol_sb = cpool.tile([P, 1], F32)
        g16_sb = cpool.tile([P, T * D], BF16)
        off16_sb = cpool.tile([P, NCH], BF16)
        iota16_sb = cpool.tile([P, P], BF16)
        zT_sb = cpool.tile([P, T * P], F32)
        wp_sb = cpool.tile([P, D], F32)
        swrow_sb = cpool.tile([1, D], F32)

        nc.sync.dma_start(
            z_sb[:].rearrange("p (t f) -> p t f", f=D),
            z0_d.ap().rearrange("(t p) f -> p t f", p=P))
        nc.sync.dma_start(w_sb[:], w_d.ap())
        nc.sync.dma_start(brow_sb[:], brow_d.ap())
        nc.sync.dma_start(gbt_sb[:], gbt_d.ap())
        nc.sync.dma_start(deg_sb[:], deg_d.ap())
        nc.sync.dma_start(msk_sb[:], msk_d.ap())
        nc.sync.dma_start(idx_sb[:], idx_d.ap())
        nc.sync.dma_start(off_sb[:], off_d.ap())
        nc.sync.dma_start(cst_sb[:], cst_d.ap())

        nc.vector.memset(ones_sb[:], 1.0)
        nc.vector.memset(onecol_sb[:], 1.0)
        nc.vector.tensor_scalar_mul(negoff_sb[:], off_sb[:], -1.0)
        nc.vector.tensor_copy(off16_sb[:], off_sb[:])
        nc.vector.tensor_copy(iota16_sb[:], iota_sb)
        nc.scalar.sqrt(dis_sb[:], deg_sb[:])
        nc.vector.reciprocal(dis_sb[:], dis_sb[:])

        pag = ctx.enter_context(tc.tile_pool(name="pagg", bufs=2, space="PSUM"))
        pzt = ctx.enter_context(tc.tile_pool(name="pzt", bufs=1, space="PSUM"))
        ph = ctx.enter_context(tc.tile_pool(name="ph", bufs=2, space="PSUM"))
        pst = ctx.enter_context(tc.tile_pool(name="pst", bufs=1, space="PSUM"))

        # replicate per-layer bias rows across partitions (rank-1 matmul)
        for li in range(L):
            bp = pag.tile([P, D], F32, tag="agg")
            nc.tensor.matmul(bp[:], ones_sb[:], brow_sb[:, li * D:(li + 1) * D],
                             start=True, stop=True)
            nc.scalar.copy(brep_sb[:, li * D:(li + 1) * D], bp[:])

        aspool = ctx.enter_context(tc.tile_pool(name="asp", bufs=2))
        a_col = aspool.tile([P, 1], F32, tag="a")
        s_col = aspool.tile([P, 1], F32, tag="s")
        nc.vector.memset(a_col[:], 1.0)
        nc.vector.memset(s_col[:], 0.0)

        ztpool = ctx.enter_context(tc.tile_pool(name="ztp", bufs=3))
        spool = ctx.enter_context(tc.tile_pool(name="sel", bufs=4))
        sqpool = ctx.enter_context(tc.tile_pool(name="sqp", bufs=3))
        msgpool = ctx.enter_context(tc.tile_pool(name="msg", bufs=4))
        smallp = ctx.enter_context(tc.tile_pool(name="small", bufs=2))

        # prologue: transpose the initial z tiles into the persistent zT buffer
        for t in range(T):
            zt_ps = pzt.tile([P, P], F32, tag="zt")
            nc.tensor.transpose(zt_ps[:], z_sb[:, t * D:(t + 1) * D],
                                ident_sb)
            nc.scalar.copy(zT_sb[:, t * P:(t + 1) * P], zt_ps[:])

        nrep = int(os.environ.get("NREP", "1"))
        for li0 in range(L * nrep):
            li = li0 % L
            wl = w_sb[:, li * D:(li + 1) * D]
            brep_l = brep_sb[:, li * D:(li + 1) * D]
            g_fullX = dram.tile([C * TS * P, D], BF16, addr_space="Shared",
                                name=f"g_fx_{li0}")
            g_fullY = dram.tile([C * TSY * P, D], BF16, addr_space="Shared",
                                name=f"g_fy_{li0}")
            st_out = dram.tile([P, 2], F32, addr_space="Shared",
                               name=f"st_out_{li0}")

            def flush_g(lo, hi, g_own_part, g_full_part):
                # convert g[:, lo:hi tiles] to bf16, publish, AllGather.  The
                # X half launches mid-phase-1 so the collective overlaps the
                # remaining tiles' matmuls.
                nc.vector.tensor_copy(g16_sb[:, lo * D:hi * D],
                                      g_sb[:, lo * D:hi * D])
                nc.sync.dma_start(
                    g_own_part[:].rearrange("(t p) f -> p t f", p=P),
                    g16_sb[:, lo * D:hi * D].rearrange("p (t f) -> p t f",
                                                       f=D))
                if "allgather" in ABLATE or "localcomm" in ABLATE:
                    nc.sync.dma_start(g_full_part[0:(hi - lo) * P, :],
                                      g_own_part[:])
                else:
                    nc.gpsimd.collective_compute(
                        "AllGather", ALU.bypass,
                        replica_groups=[list(range(C))],
                        ins=[g_own_part.opt()], outs=[g_full_part.opt()])

            # ---- phase 1: BN folded into weights:
            #   h = bn(z) @ W = z @ (a ⊙ W) + (s @ W);  g = h * dis ----
            nc.scalar.mul(wp_sb[:], wl, a_col[:])
            sw_ps = pst.tile([1, D], F32, tag="sw")
            nc.tensor.matmul(sw_ps[:], s_col[:], wl, start=True, stop=True)
            nc.scalar.copy(swrow_sb[:], sw_ps[:])
            for t in range(T):
                hp = ph.tile([P, D], F32, tag="h")
                nc.tensor.matmul(hp[:], zT_sb[:, t * P:(t + 1) * P], wp_sb[:],
                                 start=True, stop=False)
                nc.tensor.matmul(hp[:], ones_sb[:], swrow_sb[:],
                                 start=False, stop=True)
                nc.scalar.mul(g_sb[:, t * D:(t + 1) * D], hp[:],
                              dis_sb[:, t:t + 1])
                if t == TS - 1:
                    flush_g(0, TS, g_ownX, g_fullX)
            flush_g(TS, T, g_ownY, g_fullY)

            sum_ps = pst.tile([P, 1], F32, tag="sum")
            ssq_ps = pst.tile([P, 1], F32, tag="ssq")

            # ---- phase 2: gather + segment-sum + pointwise + stats ----
            for gi, grp in enumerate(groups):
                acs, acw, bcs, bcw, nA, nB = idx_base[gi]
                msg = msgpool.tile([P, MAXSLOTS, D], BF16, tag="msg")
                if "gather" in ABLATE or "agg" in ABLATE:
                    nc.vector.memset(msg[:, 0:1, :], 0.0)
                else:
                    if nA:
                        nc.gpsimd.dma_gather(
                            msg[:, 0:nA, :], g_fullX[0:C * TS * P, :],
                            idx_sb[:, acs:acs + acw], nA * P, nA * P, D,
                            single_packet=False, queue_num=gi % NSWQ)
                    if nB:
                        nc.gpsimd.dma_gather(
                            msg[:, nA:nA + nB, :], g_fullY[0:C * TSY * P, :],
                            idx_sb[:, bcs:bcs + bcw], nB * P, nB * P, D,
                            single_packet=False, queue_num=(gi + 1) % NSWQ)
                sa = 0
                sb_ = nA
                for t in grp:
                    nch = int(CA[t] + CB[t])
                    if "agg" in ABLATE:
                        nch = 0
                    agg = pag.tile([P, D], F32, tag="agg")
                    for k in range(nch):
                        if k < CA[t]:
                            cc = int(colA[t] + k)
                            slot = sa + k
                        else:
                            cc = int(colB[t] + (k - CA[t]))
                            slot = sb_ + (k - CA[t])
                        sel = spool.tile([P, P], BF16, tag="S")
                        if SMOD and k % SMOD == SMOD - 1:
                            # ACT path: S = relu(1 - (iota - off)^2)
                            nc.scalar.activation(
                                sel[:], iota_sb,
                                mybir.ActivationFunctionType.Square,
                                bias=negoff_sb[:, cc:cc + 1])
                            nc.scalar.activation(
                                sel[:], sel[:],
                                mybir.ActivationFunctionType.Relu,
                                bias=onecol_sb[:], scale=-1.0)
                        else:
                            nc.vector.tensor_single_scalar(
                                sel[:], iota16_sb[:], off_sb[:, cc:cc + 1],
                                ALU.is_equal)
                        nc.tensor.matmul(agg[:], sel[:], msg[:, slot, :],
                                         start=(k == 0), stop=(k == nch - 1))
                    sa += int(CA[t])
                    sb_ += int(CB[t])

                    zsl = z_sb[:, t * D:(t + 1) * D]
                    gsl = g_sb[:, t * D:(t + 1) * D]
                    if nch:
                        nc.vector.tensor_add(zsl, agg[:], gsl)
                        nc.vector.scalar_tensor_tensor(
                            zsl, zsl, dis_sb[:, t:t + 1], brep_l,
                            op0=ALU.mult, op1=ALU.add)
                    else:
                        nc.vector.scalar_tensor_tensor(
                            zsl, gsl, dis_sb[:, t:t + 1], brep_l,
                            op0=ALU.mult, op1=ALU.add)
                    nc.scalar.activation(zsl, zsl,
                                         mybir.ActivationFunctionType.Relu)
                    zt_ps = pzt.tile([P, P], F32, tag="zt")
                    nc.tensor.transpose(zt_ps[:], zsl, ident_sb)
                    nc.scalar.copy(zT_sb[:, t * P:(t + 1) * P], zt_ps[:])
                    sq = sqpool.tile([P, D], F32, tag="sq")
                    nc.scalar.square(sq[:], zsl)
                    if "stats" not in ABLATE:
                        nc.tensor.matmul(sum_ps[:], zsl, msk_sb[:, t:t + 1],
                                         start=(t == 0), stop=(t == T - 1),
                                         skip_group_check=True)
                        nc.tensor.matmul(ssq_ps[:], sq[:], msk_sb[:, t:t + 1],
                                         start=(t == 0), stop=(t == T - 1),
                                         skip_group_check=True)

            # ---- phase 3: BN stats AllReduce + a/s columns ----
            if "stats" in ABLATE:
                continue
            st_sb = smallp.tile([P, 2], F32, tag="st")
            nc.vector.tensor_copy(st_sb[:, 0:1], sum_ps[:])
            nc.vector.tensor_copy(st_sb[:, 1:2], ssq_ps[:])
            nc.sync.dma_start(st_in[:], st_sb[:])
            if "localcomm" in ABLATE:
                nc.sync.dma_start(st_out[0:P, :], st_in[:])
            else:
                nc.gpsimd.collective_compute(
                    "AllReduce", ALU.add,
                    replica_groups=[list(range(C))],
                    ins=[st_in.opt()], outs=[st_out.opt()])
            st2 = smallp.tile([P, 2], F32, tag="st2")
            nc.sync.dma_start(st2[:], st_out[:])
            mean = smallp.tile([P, 1], F32, tag="mean")
            ex2 = smallp.tile([P, 1], F32, tag="ex2")
            m2 = smallp.tile([P, 1], F32, tag="m2")
            var = smallp.tile([P, 1], F32, tag="var")
            sd = smallp.tile([P, 1], F32, tag="sd")
            isd = smallp.tile([P, 1], F32, tag="isd")
            tmp = smallp.tile([P, 1], F32, tag="tmp")
            nc.vector.tensor_scalar_mul(mean[:], st2[:, 0:1], 1.0 / NN)
            nc.vector.tensor_scalar_mul(ex2[:], st2[:, 1:2], 1.0 / NN)
            nc.scalar.square(m2[:], mean[:])
            nc.vector.tensor_sub(var[:], ex2[:], m2[:])
            nc.scalar.activation(sd[:], var[:],
                                 mybir.ActivationFunctionType.Sqrt,
                                 bias=eps_sb)
            nc.vector.reciprocal(isd[:], sd[:])
            a_col = aspool.tile([P, 1], F32, tag="a")
            s_col = aspool.tile([P, 1], F32, tag="s")
            nc.vector.tensor_mul(a_col[:], gbt_sb[:, li:li + 1], isd[:])
            nc.vector.tensor_mul(tmp[:], mean[:], a_col[:])
            nc.vector.tensor_sub(s_col[:], gbt_sb[:, L + li:L + li + 1], tmp[:])

        # ---- final BN apply (transposed) + output ----
        for t in range(T):
            zo_sb = ztpool.tile([P, P], F32, tag="zt")
            nc.vector.tensor_scalar(zo_sb[:], zT_sb[:, t * P:(t + 1) * P],
                                    a_col[:], s_col[:], ALU.mult, ALU.add)
            nc.sync.dma_start(zo_d[t * P:(t + 1) * P, :], zo_sb[:])


# --------------------------------------------------------------------------
# Entry points
# --------------------------------------------------------------------------

def make_in_maps(meta, per_core, Ws, bs, gammas, betas):
    Ws = np.asarray(Ws, np.float32)
    wimg = np.concatenate([Ws[li] for li in range(L)], axis=1)       # [P, L*D]
    brow = np.concatenate([np.asarray(bs[li], np.float32)
                           for li in range(L)])[None, :]             # [1, L*D]
    gbt = np.stack([np.asarray(gammas[li], np.float32) for li in range(L)]
                   + [np.asarray(betas[li], np.float32) for li in range(L)],
                   axis=1)                                           # [P, 2*L]
    consts = np.zeros((P, 2 * P + 1), np.float32)
    consts[:, 0:P] = np.arange(P, dtype=np.float32)[None, :]   # iota row
    consts[:, P:2 * P] = np.eye(P, dtype=np.float32)           # identity
    consts[:, 2 * P] = EPS
    in_maps = []
    for ci in range(C):
        pc = per_core[ci]
        in_maps.append(dict(
            z0=pc["z0"], wimg=wimg, brow=brow, gbt=gbt,
            degT=pc["degT"], mskT=pc["mskT"],
            idximg=pc["idximg"], offimg=pc["offimg"], consts=consts))
    return in_maps


def unshard(meta, outs):
    """outs: list of 8 per-core zout arrays [T*P, P] (transposed tiles)."""
    T = meta["T"]
    flat = np.stack([o.reshape(T, P, P).transpose(0, 2, 1).reshape(T * P, P)
                     for o in outs])                    # [C, R, D] row-major
    flat = flat.reshape(C * T * P, D)
    return flat[meta["pad_id"]]


def build_nc(meta):
    nc = bacc.Bacc("TRN2", target_bir_lowering=False, debug=False,
                   num_devices=C, num_swdge_queues=NSWQ)
    build(nc, meta)
    nc.compile()
    return nc


class Runner:
    """Cached PJRT executable for the SPMD bass program (mirrors
    bass2jax.run_bass_via_pjrt's multi-core branch, but reusable so repeated
    executions don't re-trace/compile)."""

    def __init__(self, nc):
        import jax
        from jax.experimental.shard_map import shard_map
        from jax.sharding import Mesh, PartitionSpec
        from concourse import bass2jax as b2j

        b2j.install_neuronx_cc_hook()
        self.nc = nc
        partition_name = (nc.partition_id_tensor.name
                          if nc.partition_id_tensor else None)
        in_names, out_names, out_avals, zero_shapes = [], [], [], []
        for alloc in nc.m.functions[0].allocations:
            if not isinstance(alloc, mybir.MemoryLocationSet):
                continue
            name = alloc.memorylocations[0].name
            if alloc.kind == "ExternalInput":
                if name != partition_name:
                    in_names.append(name)
            elif alloc.kind == "ExternalOutput":
                shape = tuple(alloc.tensor_shape)
                dtype = mybir.dt.np(alloc.dtype)
                out_names.append(name)
                out_avals.append(jax.core.ShapedArray(shape, dtype))
                zero_shapes.append((shape, dtype))
        self.in_names = list(in_names)
        self.out_names = out_names
        self.out_avals = out_avals
        self.zero_shapes = zero_shapes
        n_params = len(in_names)
        n_outs = len(out_names)
        all_in_names = in_names + out_names
        if partition_name is not None:
            all_in_names.append(partition_name)

        def _body(*args):
            operands = list(args)
            if partition_name is not None:
                operands.append(b2j.partition_id_tensor())
            outs = b2j._bass_exec_p.bind(
                *operands,
                out_avals=tuple(out_avals),
                in_names=tuple(all_in_names),
                out_names=tuple(out_names),
                lowering_input_output_aliases=(),
                sim_require_finite=True,
                sim_require_nnan=True,
                nc=nc,
            )
            return tuple(outs)

        devices = jax.devices()[:C]
        mesh = Mesh(np.asarray(devices), ("core",))
        in_specs = (PartitionSpec("core"),) * (n_params + n_outs)
        out_specs = (PartitionSpec("core"),) * n_outs
        self.sharded = jax.jit(
            shard_map(_body, mesh=mesh, in_specs=in_specs,
                      out_specs=out_specs, check_rep=False),
            donate_argnums=tuple(range(n_params, n_params + n_outs)),
            keep_unused=True,
        )
        self._jax = jax
        self._sharding = jax.sharding.NamedSharding(mesh, PartitionSpec("core"))

    def put_inputs(self, concat_in):
        return [self._jax.device_put(a, self._sharding) for a in concat_in]

    def put_zeros(self):
        return [self._jax.device_put(np.zeros((C * s[0], *s[1:]), d),
                                     self._sharding)
                for s, d in self.zero_shapes]

    def pack(self, in_maps):
        return [np.concatenate([np.asarray(m[n]) for m in in_maps], axis=0)
                for n in self.in_names]

    def run_packed(self, concat_in):
        zeros = [np.zeros((C * s[0], *s[1:]), d) for s, d in self.zero_shapes]
        out_arrs = self.sharded(*concat_in, *zeros)
        self._jax.block_until_ready(out_arrs)
        return out_arrs

    def run(self, in_maps):
        out_arrs = self.run_packed(self.pack(in_maps))
        return [
            {n: np.asarray(out_arrs[i]).reshape(C, *self.out_avals[i].shape)[c]
             for i, n in enumerate(self.out_names)}
            for c in range(C)
        ]


def kernel(x=None, edge_index=None, Ws=None, bs=None, gammas=None, betas=None):
    x = np.asarray(x, np.float32)
    meta, per_core = prep(x, np.asarray(edge_index))
    in_maps = make_in_maps(meta, per_core, Ws, bs, gammas, betas)
    nc = build_nc(meta)
    results = Runner(nc).run(in_maps)
    outs = [r["zout"] for r in results]
    return unshard(meta, outs).astype(np.float32)



# revision 27
# speedup vs baseline: 1.1646x; 1.0597x over previous
"""GCN encoder (4x GCNConv+ReLU+BatchNorm) as a Trainium2 Bass kernel on 8 cores.

Sharding: destination nodes are partitioned into 8*T degree-balanced buckets of
128 rows (one bucket = one 128-row "tile" on one core).  Per layer, each core:
  1. PE-transposes each z tile (fused with the BatchNorm apply of the previous
     layer), computes h = z @ W on PE, scales rows by dis = 1/sqrt(deg) -> g.
  2. AllGathers g into a full [NPAD, 128] DRAM table.
  3. For each dst tile, gathers the source rows of its (host-bucketed, padded)
     edges with SWDGE dma_gather, builds one-hot selector matrices on DVE
     (is_equal against an iota row) and segment-sums messages into PSUM with
     PE matmuls (out = S.T @ msgs).
  4. z' = relu(dis * (agg + g) + b); BN statistics via matmuls with a validity
     mask column; stats AllReduce'd across cores ([128, 2]).
The final BN apply happens in the transposed orientation, so the kernel output
is per-tile transposed; the host undoes the permutation and transposition.
"""

import math
import os
from contextlib import ExitStack

import numpy as np

import concourse.bass as bass
import concourse.bacc as bacc
import concourse.mybir as mybir
import concourse.tile as tile
from concourse import bass_utils

P = 128          # partitions / tile rows
D = 128          # feature dim
L = 4            # layers
C = 8            # cores
HALF = 32768     # int16 gather-index limit
EPS = 1e-5
F32 = mybir.dt.float32
BF16 = mybir.dt.bfloat16
I16 = mybir.dt.int16
ALU = mybir.AluOpType
GROUP_TILES = 4  # dst tiles per gather batch
ABLATE = set()   # debug: subset of {"agg", "gather", "stats", "allgather"}
SMOD = 0         # every SMOD-th selector build goes to ACT (0 = never; ACT latency makes this a loss)
NSWQ = 2         # SWDGE queues; gathers alternate queues to overlap desc-gen/transfer


# --------------------------------------------------------------------------
# Host-side sharding
# --------------------------------------------------------------------------

def _balance_buckets(deg, n_buckets):
    """Snake round-robin over degree-sorted nodes -> (bucket, slot) per node.
    Each bucket gets <= ceil(N / n_buckets) nodes with near-equal degree sums."""
    n = deg.shape[0]
    order = np.argsort(-deg, kind="stable")
    idx = np.arange(n)
    rounds = idx // n_buckets
    pos = idx % n_buckets
    b_of = np.where(rounds % 2 == 0, pos, n_buckets - 1 - pos)
    bucket = np.empty(n, np.int64)
    slot = np.empty(n, np.int64)
    bucket[order] = b_of
    slot[order] = rounds
    return bucket, slot


def _wrap_idx(v):
    """Wrap a flat int array into the SWDGE index image rows: img[p, s] =
    v[s*16 + p % 16], replicated across the 8 groups of 16 partitions."""
    n = v.shape[0]
    assert n % 16 == 0
    blk = v.reshape(n // 16, 16).T.astype(np.int16)   # [16, n/16]
    return np.tile(blk, (8, 1))                       # [128, n/16]


def prep(x, edge_index):
    """Shard the graph. Returns (meta, per_core_inputs: list of dicts)."""
    n_nodes, d = x.shape
    assert d == D
    src = np.asarray(edge_index[0], dtype=np.int64)
    dst = np.asarray(edge_index[1], dtype=np.int64)

    T = -(-n_nodes // (C * P))          # tiles per core
    R = T * P                           # padded rows per core
    NPAD = C * R
    assert NPAD - HALF < HALF, "second half-table must also fit int16 indices"

    deg = np.bincount(dst, minlength=n_nodes)
    bucket, slot = _balance_buckets(deg, C * T)
    assert slot.max() < P
    core_of = bucket // T
    tile_of = bucket % T
    pad_id = core_of * R + tile_of * P + slot        # padded global row id

    cnt = np.bincount(bucket, minlength=C * T).reshape(C, T)

    TS = -(-T // 2)                     # tiles per core in table X
    TSY = T - TS                        # tiles per core in table Y
    assert C * TS * P <= 32768 and C * TSY * P <= 32768

    e_core = core_of[dst]
    e_tile = tile_of[dst]
    e_off = slot[dst]
    src_core = core_of[src]
    src_tile = tile_of[src]
    src_slot = slot[src]
    e_half = (src_tile >= TS).astype(np.int64)    # 0 -> table X, 1 -> table Y
    e_srcp = np.where(e_half == 0,
                      src_core * (TS * P) + src_tile * P + src_slot,
                      src_core * (TSY * P) + (src_tile - TS) * P + src_slot)

    # chunk budgets per (tile, half): max over cores
    key = ((e_core * T) + e_tile) * 2 + e_half
    ecnt = np.bincount(key, minlength=C * T * 2).reshape(C, T, 2)
    CA = -(-ecnt[:, :, 0].max(axis=0) // P)          # [T]
    CB = -(-ecnt[:, :, 1].max(axis=0) // P)

    groups = [list(range(s, min(s + GROUP_TILES, T)))
              for s in range(0, T, GROUP_TILES)]

    # chunk column bases (t-major, A chunks then B chunks per tile)
    colA = np.zeros(T, np.int64)
    colB = np.zeros(T, np.int64)
    c = 0
    for t in range(T):
        colA[t] = c
        c += CA[t]
        colB[t] = c
        c += CB[t]
    NCH = int(c)

    # idx image column layout per group: A segment then B segment
    idx_base = []
    ic = 0
    for grp in groups:
        nA = int(sum(CA[t] for t in grp))
        nB = int(sum(CB[t] for t in grp))
        idx_base.append((ic, nA * 8, ic + nA * 8, nB * 8, nA, nB))
        ic += (nA + nB) * 8
    IDXC = ic
    MAXSLOTS = max(a[4] + a[5] for a in idx_base)

    meta = dict(N=n_nodes, T=T, R=R, NPAD=NPAD, CA=CA, CB=CB,
                groups=groups, colA=colA, colB=colB, NCH=NCH,
                idx_base=idx_base, IDXC=IDXC, MAXSLOTS=MAXSLOTS,
                pad_id=pad_id, TS=TS, TSY=TSY)

    # ---------------- per-core images ----------------
    per_core = []
    eorder = np.lexsort((e_half, e_tile, e_core))    # sort edges
    s_core = e_core[eorder]
    s_tile = e_tile[eorder]
    s_half = e_half[eorder]
    s_srcp = e_srcp[eorder]
    s_off = e_off[eorder]

    # start offset of each (core, tile, half) run in the sorted arrays
    runkey = ((s_core * T) + s_tile) * 2 + s_half
    starts = np.searchsorted(runkey, np.arange(C * T * 2))

    for ci in range(C):
        z0 = np.zeros((R, D), np.float32)
        nodes_c = np.where(core_of == ci)[0]
        z0[tile_of[nodes_c] * P + slot[nodes_c]] = x[nodes_c]

        degT = np.ones((P, T), np.float32)
        mskT = np.zeros((P, T), np.float32)
        tt = tile_of[nodes_c]
        ss = slot[nodes_c]
        degT[ss, tt] = deg[nodes_c] + 1.0
        mskT[ss, tt] = 1.0

        offimg = np.full((P, NCH), -1.0, np.float32)
        idx_img = np.zeros((P, IDXC), np.int16)

        for gi, grp in enumerate(groups):
            for hi in (0, 1):
                vs = []
                for t in grp:
                    budget = (CA[t] if hi == 0 else CB[t]) * P
                    if budget == 0:
                        continue
                    k0 = ((ci * T) + t) * 2 + hi
                    a = starts[k0]
                    b = starts[k0 + 1] if k0 + 1 < C * T * 2 else len(runkey)
                    srcs = s_srcp[a:b]
                    offs = s_off[a:b]
                    assert len(srcs) <= budget
                    v = np.zeros(budget, np.int64)
                    v[:len(srcs)] = srcs
                    o = np.full(budget, -1.0, np.float32)
                    o[:len(srcs)] = offs
                    vs.append(v)
                    cb = (colA[t] if hi == 0 else colB[t])
                    nch_t = budget // P
                    offimg[:, cb:cb + nch_t] = o.reshape(nch_t, P).T
                if vs:
                    vflat = np.concatenate(vs)
                    cs = idx_base[gi][0] if hi == 0 else idx_base[gi][2]
                    cw = idx_base[gi][1] if hi == 0 else idx_base[gi][3]
                    assert vflat.shape[0] // 16 == cw  # cols match
                    idx_img[:, cs:cs + cw] = _wrap_idx(vflat)

        per_core.append(dict(z0=z0, degT=degT, mskT=mskT,
                             idximg=idx_img, offimg=offimg))
    return meta, per_core


# --------------------------------------------------------------------------
# Device program
# --------------------------------------------------------------------------

def build(nc, meta):
    T = meta["T"]
    NPAD = meta["NPAD"]
    NN = meta["N"]
    CA, CB = meta["CA"], meta["CB"]
    colA, colB = meta["colA"], meta["colB"]
    groups = meta["groups"]
    idx_base = meta["idx_base"]
    IDXC, NCH, MAXSLOTS = meta["IDXC"], meta["NCH"], meta["MAXSLOTS"]

    z0_d = nc.dram_tensor("z0", [T * P, D], F32, kind="ExternalInput")
    w_d = nc.dram_tensor("wimg", [P, L * D], F32, kind="ExternalInput")
    brow_d = nc.dram_tensor("brow", [1, L * D], F32, kind="ExternalInput")
    gbt_d = nc.dram_tensor("gbt", [P, 2 * L], F32, kind="ExternalInput")
    deg_d = nc.dram_tensor("degT", [P, T], F32, kind="ExternalInput")
    msk_d = nc.dram_tensor("mskT", [P, T], F32, kind="ExternalInput")
    idx_d = nc.dram_tensor("idximg", [P, IDXC], I16, kind="ExternalInput")
    off_d = nc.dram_tensor("offimg", [P, NCH], F32, kind="ExternalInput")
    cst_d = nc.dram_tensor("consts", [P, 2 * P + 1], F32, kind="ExternalInput")
    zo_d = nc.dram_tensor("zout", [T * P, P], F32, kind="ExternalOutput")

    TS, TSY = meta["TS"], meta["TSY"]

    with tile.TileContext(nc) as tc, ExitStack() as ctx:
        dram = ctx.enter_context(tc.tile_pool(name="dram", bufs=1, space="DRAM"))
        g_ownX = dram.tile([TS * P, D], BF16)
        g_ownY = dram.tile([TSY * P, D], BF16)
        st_in = dram.tile([P, 2], F32)

        cpool = ctx.enter_context(tc.tile_pool(name="const", bufs=1))
        z_sb = cpool.tile([P, T * D], F32)
        g_sb = cpool.tile([P, T * D], F32)
        w_sb = cpool.tile([P, L * D], F32)
        brow_sb = cpool.tile([1, L * D], F32)
        brep_sb = cpool.tile([P, L * D], F32)
        gbt_sb = cpool.tile([P, 2 * L], F32)
        deg_sb = cpool.tile([P, T], F32)
        msk_sb = cpool.tile([P, T], F32)
        dis_sb = cpool.tile([P, T], F32)
        idx_sb = cpool.tile([P, IDXC], I16)
        off_sb = cpool.tile([P, NCH], F32)
        cst_sb = cpool.tile([P, 2 * P + 1], F32)
        iota_sb = cst_sb[:, 0:P]
        ident_sb = cst_sb[:, P:2 * P]
        eps_sb = cst_sb[:, 2 * P:2 * P + 1]
        ones_sb = cpool.tile([1, P], F32)
        negoff_sb = cpool.tile([P, NCH], F32)
        onecol_sb = cpool.tile([P, 1], F32)
        g16_sb = cpool.tile([P, T * D], BF16)
        iota16_sb = cpool.tile([P, P], BF16)
        zT_sb = cpool.tile([P, T * P], F32)
        wp_sb = cpool.tile([P, D], F32)
        swrow_sb = cpool.tile([1, D], F32)

        nc.sync.dma_start(
            z_sb[:].rearrange("p (t f) -> p t f", f=D),
            z0_d.ap().rearrange("(t p) f -> p t f", p=P))
        nc.sync.dma_start(w_sb[:], w_d.ap())
        nc.sync.dma_start(brow_sb[:], brow_d.ap())
        nc.sync.dma_start(gbt_sb[:], gbt_d.ap())
        nc.sync.dma_start(deg_sb[:], deg_d.ap())
        nc.sync.dma_start(msk_sb[:], msk_d.ap())
        nc.sync.dma_start(idx_sb[:], idx_d.ap())
        nc.sync.dma_start(off_sb[:], off_d.ap())
        nc.sync.dma_start(cst_sb[:], cst_d.ap())

        nc.vector.memset(ones_sb[:], 1.0)
        nc.vector.memset(onecol_sb[:], 1.0)
        nc.vector.tensor_scalar_mul(negoff_sb[:], off_sb[:], -1.0)
        nc.vector.tensor_copy(iota16_sb[:], iota_sb)
        nc.scalar.sqrt(dis_sb[:], deg_sb[:])
        nc.vector.reciprocal(dis_sb[:], dis_sb[:])

        pag = ctx.enter_context(tc.tile_pool(name="pagg", bufs=2, space="PSUM"))
        pzt = ctx.enter_context(tc.tile_pool(name="pzt", bufs=1, space="PSUM"))
        ph = ctx.enter_context(tc.tile_pool(name="ph", bufs=2, space="PSUM"))
        pst = ctx.enter_context(tc.tile_pool(name="pst", bufs=1, space="PSUM"))

        # replicate per-layer bias rows across partitions (rank-1 matmul)
        for li in range(L):
            bp = pag.tile([P, D], F32, tag="agg")
            nc.tensor.matmul(bp[:], ones_sb[:], brow_sb[:, li * D:(li + 1) * D],
                             start=True, stop=True)
            nc.scalar.copy(brep_sb[:, li * D:(li + 1) * D], bp[:])

        aspool = ctx.enter_context(tc.tile_pool(name="asp", bufs=2))
        a_col = aspool.tile([P, 1], F32, tag="a")
        s_col = aspool.tile([P, 1], F32, tag="s")
        nc.vector.memset(a_col[:], 1.0)
        nc.vector.memset(s_col[:], 0.0)

        ztpool = ctx.enter_context(tc.tile_pool(name="ztp", bufs=3))
        spool = ctx.enter_context(tc.tile_pool(name="sel", bufs=4))
        sqpool = ctx.enter_context(tc.tile_pool(name="sqp", bufs=3))
        msgpool = ctx.enter_context(tc.tile_pool(name="msg", bufs=4))
        smallp = ctx.enter_context(tc.tile_pool(name="small", bufs=2))

        # prologue: transpose the initial z tiles into the persistent zT buffer
        for t in range(T):
            zt_ps = pzt.tile([P, P], F32, tag="zt")
            nc.tensor.transpose(zt_ps[:], z_sb[:, t * D:(t + 1) * D],
                                ident_sb)
            nc.scalar.copy(zT_sb[:, t * P:(t + 1) * P], zt_ps[:])

        nrep = int(os.environ.get("NREP", "1"))
        for li0 in range(L * nrep):
            li = li0 % L
            wl = w_sb[:, li * D:(li + 1) * D]
            brep_l = brep_sb[:, li * D:(li + 1) * D]
            g_fullX = dram.tile([C * TS * P, D], BF16, addr_space="Shared",
                                name=f"g_fx_{li0}")
            g_fullY = dram.tile([C * TSY * P, D], BF16, addr_space="Shared",
                                name=f"g_fy_{li0}")
            st_out = dram.tile([P, 2], F32, addr_space="Shared",
                               name=f"st_out_{li0}")

            def flush_g(lo, hi, g_own_part, g_full_part):
                # convert g[:, lo:hi tiles] to bf16, publish, AllGather.  The
                # X half launches mid-phase-1 so the collective overlaps the
                # remaining tiles' matmuls.
                nc.vector.tensor_copy(g16_sb[:, lo * D:hi * D],
                                      g_sb[:, lo * D:hi * D])
                nc.sync.dma_start(
                    g_own_part[:].rearrange("(t p) f -> p t f", p=P),
                    g16_sb[:, lo * D:hi * D].rearrange("p (t f) -> p t f",
                                                       f=D))
                if "allgather" in ABLATE or "localcomm" in ABLATE:
                    nc.sync.dma_start(g_full_part[0:(hi - lo) * P, :],
                                      g_own_part[:])
                else:
                    nc.gpsimd.collective_compute(
                        "AllGather", ALU.bypass,
                        replica_groups=[list(range(C))],
                        ins=[g_own_part.opt()], outs=[g_full_part.opt()])

            # ---- phase 1: BN folded into weights:
            #   h = bn(z) @ W = z @ (a ⊙ W) + (s @ W);  g = h * dis ----
            nc.scalar.mul(wp_sb[:], wl, a_col[:])
            sw_ps = pst.tile([1, D], F32, tag="sw")
            nc.tensor.matmul(sw_ps[:], s_col[:], wl, start=True, stop=True)
            nc.scalar.copy(swrow_sb[:], sw_ps[:])
            for t in range(T):
                hp = ph.tile([P, D], F32, tag="h")
                nc.tensor.matmul(hp[:], zT_sb[:, t * P:(t + 1) * P], wp_sb[:],
                                 start=True, stop=False)
                nc.tensor.matmul(hp[:], ones_sb[:], swrow_sb[:],
                                 start=False, stop=True)
                nc.scalar.mul(g_sb[:, t * D:(t + 1) * D], hp[:],
                              dis_sb[:, t:t + 1])
                if t == TS - 1:
                    flush_g(0, TS, g_ownX, g_fullX)
            flush_g(TS, T, g_ownY, g_fullY)

            sum_ps = pst.tile([P, 1], F32, tag="sum")
            ssq_ps = pst.tile([P, 1], F32, tag="ssq")

            # ---- phase 2: gather + segment-sum + pointwise + stats ----
            for gi, grp in enumerate(groups):
                acs, acw, bcs, bcw, nA, nB = idx_base[gi]
                msg = msgpool.tile([P, MAXSLOTS, D], BF16, tag="msg")
                if "gather" in ABLATE or "agg" in ABLATE:
                    nc.vector.memset(msg[:, 0:1, :], 0.0)
                else:
                    if nA:
                        nc.gpsimd.dma_gather(
                            msg[:, 0:nA, :], g_fullX[0:C * TS * P, :],
                            idx_sb[:, acs:acs + acw], nA * P, nA * P, D,
                            single_packet=False, queue_num=gi % NSWQ)
                    if nB:
                        nc.gpsimd.dma_gather(
                            msg[:, nA:nA + nB, :], g_fullY[0:C * TSY * P, :],
                            idx_sb[:, bcs:bcs + bcw], nB * P, nB * P, D,
                            single_packet=False, queue_num=(gi + 1) % NSWQ)
                sa = 0
                sb_ = nA
                for t in grp:
                    nch = int(CA[t] + CB[t])
                    if "agg" in ABLATE:
                        nch = 0
                    agg = pag.tile([P, D], F32, tag="agg")
                    for k in range(nch):
                        if k < CA[t]:
                            cc = int(colA[t] + k)
                            slot = sa + k
                        else:
                            cc = int(colB[t] + (k - CA[t]))
                            slot = sb_ + (k - CA[t])
                        sel = spool.tile([P, P], BF16, tag="S")
                        if SMOD and k % SMOD == SMOD - 1:
                            # ACT path: S = relu(1 - (iota - off)^2)
                            nc.scalar.activation(
                                sel[:], iota_sb,
                                mybir.ActivationFunctionType.Square,
                                bias=negoff_sb[:, cc:cc + 1])
                            nc.scalar.activation(
                                sel[:], sel[:],
                                mybir.ActivationFunctionType.Relu,
                                bias=onecol_sb[:], scale=-1.0)
                        else:
                            nc.vector.tensor_single_scalar(
                                sel[:], iota16_sb[:], off_sb[:, cc:cc + 1],
                                ALU.is_equal)
                        nc.tensor.matmul(agg[:], sel[:], msg[:, slot, :],
                                         start=(k == 0), stop=(k == nch - 1))
                    sa += int(CA[t])
                    sb_ += int(CB[t])

                    zsl = z_sb[:, t * D:(t + 1) * D]
                    gsl = g_sb[:, t * D:(t + 1) * D]
                    if nch:
                        nc.vector.tensor_add(zsl, agg[:], gsl)
                        nc.vector.scalar_tensor_tensor(
                            zsl, zsl, dis_sb[:, t:t + 1], brep_l,
                            op0=ALU.mult, op1=ALU.add)
                    else:
                        nc.vector.scalar_tensor_tensor(
                            zsl, gsl, dis_sb[:, t:t + 1], brep_l,
                            op0=ALU.mult, op1=ALU.add)
                    nc.scalar.activation(zsl, zsl,
                                         mybir.ActivationFunctionType.Relu)
                    zt_ps = pzt.tile([P, P], F32, tag="zt")
                    nc.tensor.transpose(zt_ps[:], zsl, ident_sb)
                    nc.scalar.copy(zT_sb[:, t * P:(t + 1) * P], zt_ps[:])
                    sq = sqpool.tile([P, D], F32, tag="sq")
                    nc.scalar.square(sq[:], zsl)
                    if "stats" not in ABLATE:
                        nc.tensor.matmul(sum_ps[:], zsl, msk_sb[:, t:t + 1],
                                         start=(t == 0), stop=(t == T - 1),
                                         skip_group_check=True)
                        nc.tensor.matmul(ssq_ps[:], sq[:], msk_sb[:, t:t + 1],
                                         start=(t == 0), stop=(t == T - 1),
                                         skip_group_check=True)

            # ---- phase 3: BN stats AllReduce + a/s columns ----
            if "stats" in ABLATE:
                continue
            st_sb = smallp.tile([P, 2], F32, tag="st")
            nc.vector.tensor_copy(st_sb[:, 0:1], sum_ps[:])
            nc.vector.tensor_copy(st_sb[:, 1:2], ssq_ps[:])
            nc.sync.dma_start(st_in[:], st_sb[:])
            if "localcomm" in ABLATE:
                nc.sync.dma_start(st_out[0:P, :], st_in[:])
            else:
                nc.gpsimd.collective_compute(
                    "AllReduce", ALU.add,
                    replica_groups=[list(range(C))],
                    ins=[st_in.opt()], outs=[st_out.opt()])
            st2 = smallp.tile([P, 2], F32, tag="st2")
            nc.sync.dma_start(st2[:], st_out[:])
            mean = smallp.tile([P, 1], F32, tag="mean")
            ex2 = smallp.tile([P, 1], F32, tag="ex2")
            m2 = smallp.tile([P, 1], F32, tag="m2")
            var = smallp.tile([P, 1], F32, tag="var")
            sd = smallp.tile([P, 1], F32, tag="sd")
            isd = smallp.tile([P, 1], F32, tag="isd")
            tmp = smallp.tile([P, 1], F32, tag="tmp")
            nc.vector.tensor_scalar_mul(mean[:], st2[:, 0:1], 1.0 / NN)
            nc.vector.tensor_scalar_mul(ex2[:], st2[:, 1:2], 1.0 / NN)
            nc.scalar.square(m2[:], mean[:])
            nc.vector.tensor_sub(var[:], ex2[:], m2[:])
            nc.scalar.activation(sd[:], var[:],
                                 mybir.ActivationFunctionType.Sqrt,
                                 bias=eps_sb)
            nc.vector.reciprocal(isd[:], sd[:])
            a_col = aspool.tile([P, 1], F32, tag="a")
            s_col = aspool.tile([P, 1], F32, tag="s")
            nc.vector.tensor_mul(a_col[:], gbt_sb[:, li:li + 1], isd[:])
            nc.vector.tensor_mul(tmp[:], mean[:], a_col[:])
            nc.vector.tensor_sub(s_col[:], gbt_sb[:, L + li:L + li + 1], tmp[:])

        # ---- final BN apply (transposed) + output ----
        for t in range(T):
            zo_sb = ztpool.tile([P, P], F32, tag="zt")
            nc.vector.tensor_scalar(zo_sb[:], zT_sb[:, t * P:(t + 1) * P],
                                    a_col[:], s_col[:], ALU.mult, ALU.add)
            nc.sync.dma_start(zo_d[t * P:(t + 1) * P, :], zo_sb[:])


# --------------------------------------------------------------------------
# Entry points
# --------------------------------------------------------------------------

def make_in_maps(meta, per_core, Ws, bs, gammas, betas):
    Ws = np.asarray(Ws, np.float32)
    wimg = np.concatenate([Ws[li] for li in range(L)], axis=1)       # [P, L*D]
    brow = np.concatenate([np.asarray(bs[li], np.float32)
                           for li in range(L)])[None, :]             # [1, L*D]
    gbt = np.stack([np.asarray(gammas[li], np.float32) for li in range(L)]
                   + [np.asarray(betas[li], np.float32) for li in range(L)],
                   axis=1)                                           # [P, 2*L]
    consts = np.zeros((P, 2 * P + 1), np.float32)
    consts[:, 0:P] = np.arange(P, dtype=np.float32)[None, :]   # iota row
    consts[:, P:2 * P] = np.eye(P, dtype=np.float32)           # identity
    consts[:, 2 * P] = EPS
    in_maps = []
    for ci in range(C):
        pc = per_core[ci]
        in_maps.append(dict(
            z0=pc["z0"], wimg=wimg, brow=brow, gbt=gbt,
            degT=pc["degT"], mskT=pc["mskT"],
            idximg=pc["idximg"], offimg=pc["offimg"], consts=consts))
    return in_maps


def unshard(meta, outs):
    """outs: list of 8 per-core zout arrays [T*P, P] (transposed tiles)."""
    T = meta["T"]
    flat = np.stack([o.reshape(T, P, P).transpose(0, 2, 1).reshape(T * P, P)
                     for o in outs])                    # [C, R, D] row-major
    flat = flat.reshape(C * T * P, D)
    return flat[meta["pad_id"]]


def build_nc(meta):
    nc = bacc.Bacc("TRN2", target_bir_lowering=False, debug=False,
                   num_devices=C, num_swdge_queues=NSWQ)
    build(nc, meta)
    nc.compile()
    return nc


class Runner:
    """Cached PJRT executable for the SPMD bass program (mirrors
    bass2jax.run_bass_via_pjrt's multi-core branch, but reusable so repeated
    executions don't re-trace/compile)."""

    def __init__(self, nc):
        import jax
        from jax.experimental.shard_map import shard_map
        from jax.sharding import Mesh, PartitionSpec
        from concourse import bass2jax as b2j

        b2j.install_neuronx_cc_hook()
        self.nc = nc
        partition_name = (nc.partition_id_tensor.name
                          if nc.partition_id_tensor else None)
        in_names, out_names, out_avals, zero_shapes = [], [], [], []
        for alloc in nc.m.functions[0].allocations:
            if not isinstance(alloc, mybir.MemoryLocationSet):
                continue
            name = alloc.memorylocations[0].name
            if alloc.kind == "ExternalInput":
                if name != partition_name:
                    in_names.append(name)
            elif alloc.kind == "ExternalOutput":
                shape = tuple(alloc.tensor_shape)
                dtype = mybir.dt.np(alloc.dtype)
                out_names.append(name)
                out_avals.append(jax.core.ShapedArray(shape, dtype))
                zero_shapes.append((shape, dtype))
        self.in_names = list(in_names)
        self.out_names = out_names
        self.out_avals = out_avals
        self.zero_shapes = zero_shapes
        n_params = len(in_names)
        n_outs = len(out_names)
        all_in_names = in_names + out_names
        if partition_name is not None:
            all_in_names.append(partition_name)

        def _body(*args):
            operands = list(args)
            if partition_name is not None:
                operands.append(b2j.partition_id_tensor())
            outs = b2j._bass_exec_p.bind(
                *operands,
                out_avals=tuple(out_avals),
                in_names=tuple(all_in_names),
                out_names=tuple(out_names),
                lowering_input_output_aliases=(),
                sim_require_finite=True,
                sim_require_nnan=True,
                nc=nc,
            )
            return tuple(outs)

        devices = jax.devices()[:C]
        mesh = Mesh(np.asarray(devices), ("core",))
        in_specs = (PartitionSpec("core"),) * (n_params + n_outs)
        out_specs = (PartitionSpec("core"),) * n_outs
        self.sharded = jax.jit(
            shard_map(_body, mesh=mesh, in_specs=in_specs,
                      out_specs=out_specs, check_rep=False),
            donate_argnums=tuple(range(n_params, n_params + n_outs)),
            keep_unused=True,
        )
        self._jax = jax
        self._sharding = jax.sharding.NamedSharding(mesh, PartitionSpec("core"))

    def put_inputs(self, concat_in):
        return [self._jax.device_put(a, self._sharding) for a in concat_in]

    def put_zeros(self):
        return [self._jax.device_put(np.zeros((C * s[0], *s[1:]), d),
                                     self._sharding)
                for s, d in self.zero_shapes]

    def pack(self, in_maps):
        return [np.concatenate([np.asarray(m[n]) for m in in_maps], axis=0)
                for n in self.in_names]

    def run_packed(self, concat_in):
        zeros = [np.zeros((C * s[0], *s[1:]), d) for s, d in self.zero_shapes]
        out_arrs = self.sharded(*concat_in, *zeros)
        self._jax.block_until_ready(out_arrs)
        return out_arrs

    def run(self, in_maps):
        out_arrs = self.run_packed(self.pack(in_maps))
        return [
            {n: np.asarray(out_arrs[i]).reshape(C, *self.out_avals[i].shape)[c]
             for i, n in enumerate(self.out_names)}
            for c in range(C)
        ]


def kernel(x=None, edge_index=None, Ws=None, bs=None, gammas=None, betas=None):
    x = np.asarray(x, np.float32)
    meta, per_core = prep(x, np.asarray(edge_index))
    in_maps = make_in_maps(meta, per_core, Ws, bs, gammas, betas)
    nc = build_nc(meta)
    results = Runner(nc).run(in_maps)
    outs = [r["zout"] for r in results]
    return unshard(meta, outs).astype(np.float32)

